# revision 1
# baseline (speedup 1.0000x reference)
"""Trainium2 Bass kernel for nn_DrugResponsePrior (embedding_lookup).

Spec guarantees: cell_map < 100, is_missing in {0,1}, drug_map < 256.  So each
row's result depends only on cs = cell_map[idx]+100*is_missing[idx] (200
states) and dm = drug_map[tidx] (256 drugs).

Fully data-parallel strategy (8 cores, 8192 samples each, no collectives):
  1. Build CS = cell_map + 100*is_missing as a uint8 table (and drug_map as
     uint8) in DRAM; reload them in a 16-slab SBUF layout (partition p holds
     entries [(p%16)*16384, ...)).
  2. Look up cs/dm per sample with GPSIMD indirect_copy (per-16-partition
     group index lists, offsets = idx & 16383); resolve the 16-way slab
     ambiguity with a one-hot mask (built from idx>>14 via a small
     group-broadcast matmul + iota compare) and a group-reduce matmul.
  3. Precompute A = l2n(cell-embedding table) @ Wf1c + bf1 ([200,200]) and
     Bd = l2n(drug_emb) @ Wf1d ([256,200]) once, stored bf16.
  4. Per sample, build bf16 one-hot matrices over cs (200) / dm (256) and run
     the MLP with bf16 matmuls (fp32 PSUM accumulate):
     h1 = relu(A^T Sc + Bd^T Sd), h2 = relu(Wf2^T h1 + bf2),
     fwd = Wf3^T h2 + bf3 (bias matmul), mu = cumsum-of-softplus via one
     [9,9] matmul per chunk producing mu^T; the host transposes.
  Chunk tails (softplus, mu) are emitted one chunk late so each engine FIFO
  stays dependency-forward and chunks pipeline.

All reference math runs on device; the host only reshapes/transposes/casts
inputs and slices idx/tidx (pure index arithmetic: & 16383, >> 14).
"""
import sys

if "/opt/trn_rl_repo" not in sys.path:
    sys.path.insert(0, "/opt/trn_rl_repo")

import numpy as np
import ml_dtypes

import concourse.bass as bass
import concourse.mybir as mybir
import concourse.tile as tile
from concourse.bass_utils import run_bass_kernel_spmd

f32 = mybir.dt.float32
bf16 = mybir.dt.bfloat16
i32 = mybir.dt.int32
u16 = mybir.dt.uint16
u8 = mybir.dt.uint8
np_bf16 = ml_dtypes.bfloat16

B = 65536
R = 262144
NDRUG = 256
NFEAT = 1024
CEMB = 1024
DEMB = 128
HID = 200
NDOSES = 9
NCORES = 8

BS = B // NCORES            # 8192 samples per core
P = 128
NG = 8                      # gpsimd groups (16 partitions each)
GS = BS // NG               # 1024 samples per group
SLAB = R // 16              # 16384 entries per slab partition
NCHUNK = BS // 512          # 16 chunks of 512 samples
EPS = 1e-12

_NC_CACHE = {}


def _split_sync_waits(nc, limit=1):
    """This walrus accepts at most one sync-wait per instruction; hoist excess
    waits onto same-engine NoOps inserted just before."""
    ctr = 0
    for bb in nc.main_func.blocks:
        new_list = []
        for inst in bb.instructions:
            si = inst.sync_info
            if si is not None and si.on_wait and len(si.on_wait) > limit:
                waits = list(si.on_wait)
                head, tail = waits[:-limit], waits[-limit:]
                for j in range(0, len(head), limit):
                    nop = mybir.InstNoOp(name=f"waitnop-{ctr}", engine=inst.engine)
                    ctr += 1
                    nop.sync_info = mybir.SyncInfo(
                        on_wait=list(head[j : j + limit]), on_update=[]
                    )
                    new_list.append(nop)
                inst.sync_info = mybir.SyncInfo(
                    on_wait=list(tail),
                    on_update=list(si.on_update) if si.on_update else [],
                )
            new_list.append(inst)
        bb.instructions[:] = new_list
    return nc


def build_nc(split_waits=True):
    nc = bass.Bass(num_devices=NCORES)
    AF = mybir.ActivationFunctionType
    ALU = mybir.AluOpType

    # ---------------- kernel I/O ----------------
    u_idx = nc.dram_tensor("u_idx", [P, GS // 16], u16, kind="ExternalInput")
    u_tidx = nc.dram_tensor("u_tidx", [P, GS // 16], u16, kind="ExternalInput")
    q_idx = nc.dram_tensor("q_idx", [NG, GS], bf16, kind="ExternalInput")
    q_tidx = nc.dram_tensor("q_tidx", [NG, GS], bf16, kind="ExternalInput")
    cell_map = nc.dram_tensor("cell_map", [R], u8, kind="ExternalInput")
    is_missing = nc.dram_tensor("is_missing", [R], u8, kind="ExternalInput")
    drug_map = nc.dram_tensor("drug_map", [R], u8, kind="ExternalInput")
    cfT = nc.dram_tensor("cfT", [NFEAT, 100], bf16, kind="ExternalInput")
    me_in = nc.dram_tensor("me_in", [100, CEMB], f32, kind="ExternalInput")
    drug_emb = nc.dram_tensor("drug_emb", [NDRUG, DEMB], f32, kind="ExternalInput")
    drug_embT = nc.dram_tensor("drug_embT", [DEMB, NDRUG], bf16, kind="ExternalInput")
    W1 = nc.dram_tensor("W1", [NFEAT, CEMB], bf16, kind="ExternalInput")
    b1 = nc.dram_tensor("b1", [CEMB], f32, kind="ExternalInput")
    Wf1c = nc.dram_tensor("Wf1c", [CEMB, HID], bf16, kind="ExternalInput")
    Wf1d = nc.dram_tensor("Wf1d", [DEMB, HID], bf16, kind="ExternalInput")
    bf1 = nc.dram_tensor("bf1", [HID], f32, kind="ExternalInput")
    Wf2 = nc.dram_tensor("Wf2", [HID, HID], bf16, kind="ExternalInput")
    bf2 = nc.dram_tensor("bf2", [HID], f32, kind="ExternalInput")
    Wf3a0 = nc.dram_tensor("Wf3a0", [P, NDOSES - 1], bf16, kind="ExternalInput")
    Wf3a1p = nc.dram_tensor("Wf3a1p", [P, NDOSES - 1], bf16, kind="ExternalInput")
    Mb0 = nc.dram_tensor("Mb0", [P, NDOSES], bf16, kind="ExternalInput")
    Mb1p = nc.dram_tensor("Mb1p", [P, NDOSES], bf16, kind="ExternalInput")
    onesrow = nc.dram_tensor("onesrow", [1, 512], bf16, kind="ExternalInput")
    mu9_s = nc.dram_tensor("mu9_s", [NDOSES, BS], f32, kind="ExternalOutput")

    # inline constants (input-value independent)
    grp_bc = nc.inline_tensor(  # [8, 128] group-broadcast lhsT
        np.array([[1.0 if (m // 16) == g else 0.0 for m in range(P)]
                  for g in range(NG)], np_bf16), name="grp_bc")
    grp_rd = nc.inline_tensor(  # [128, 8] group-reduce lhsT
        np.array([[1.0 if (k // 16) == g else 0.0 for g in range(NG)]
                  for k in range(P)], np_bf16), name="grp_rd")
    qi_const = nc.inline_tensor(
        (np.arange(P, dtype=np.float32).reshape(P, 1) % 16), name="qi_const")
    # [128, 1024] compare table: cols 0:512 = p, cols 512:1024 = p + 128
    iota2np = np.concatenate(
        [np.tile(np.arange(P, dtype=np.float32).reshape(P, 1), (1, 512)),
         np.tile((np.arange(P, dtype=np.float32) + P).reshape(P, 1), (1, 512))],
        axis=1).astype(np_bf16)
    iota2 = nc.inline_tensor(iota2np, name="iota2")
    # mu softplus part: mu^T = L8^T @ gb8 (+ base via Mb^T h2 + bf3[0])
    # L8[k,o] = 1 iff dose k+1 contributes to output o (k+1 <= o)
    L8np = np.triu(np.ones((NDOSES - 1, NDOSES), np.float32), 1).astype(np_bf16)
    L8 = nc.inline_tensor(L8np, name="L8")

    # internal DRAM for uint8 lookup tables + value rows
    CS8 = nc.dram_tensor("CS8", [R], u8)
    cs_rowd = [nc.dram_tensor(f"cs_rowd{t}", [NG * 512], bf16)
               for t in range(2)]
    dm_rowd = [nc.dram_tensor(f"dm_rowd{t}", [NG * 512], bf16)
               for t in range(2)]

    with tile.TileContext(nc) as tc, \
            tc.tile_pool(name="sb", bufs=1) as sb, \
            tc.tile_pool(name="sbw", bufs=1) as sbw:

        # ======== constants / params to SBUF ========
        grp_bc_sb = sbw.tile([NG, P], bf16)
        nc.sync.dma_start(out=grp_bc_sb[:], in_=grp_bc[:])
        grp_rd_sb = sbw.tile([P, NG], bf16)
        nc.sync.dma_start(out=grp_rd_sb[:], in_=grp_rd[:])
        qi_sb = sbw.tile([P, 1], f32)
        nc.sync.dma_start(out=qi_sb[:], in_=qi_const[:])
        iota2_sb = sbw.tile([P, 1024], bf16)
        nc.sync.dma_start(out=iota2_sb[:], in_=iota2[:])
        L8_sb = sbw.tile([NDOSES - 1, NDOSES], bf16)
        nc.sync.dma_start(out=L8_sb[:], in_=L8[:])
        ones_c100 = sbw.tile([1, 100], f32)
        nc.vector.memset(ones_c100[:], 1.0)
        ones_c128 = sbw.tile([1, P], f32)
        nc.vector.memset(ones_c128[:], 1.0)

        u_idx_sb = sb.tile([P, GS // 16], u16)
        nc.sync.dma_start(out=u_idx_sb[:], in_=u_idx[:])
        u_tidx_sb = sb.tile([P, GS // 16], u16)
        nc.sync.dma_start(out=u_tidx_sb[:], in_=u_tidx[:])
        q_idx_sb = sb.tile([NG, GS], bf16)
        nc.sync.dma_start(out=q_idx_sb[:], in_=q_idx[:])
        q_tidx_sb = sb.tile([NG, GS], bf16)
        nc.sync.dma_start(out=q_tidx_sb[:], in_=q_tidx[:])

        # ======== phase 1: build CS8/DM8 and load slabs ========
        with tc.tile_pool(name="sbx", bufs=1) as sbx:
            cmv = sbx.tile([P, R // P], u8)
            miv = sbx.tile([P, R // P], u8)
            nc.sync.dma_start(out=cmv[:], in_=cell_map[:].rearrange("(p a) -> p a", p=P))
            nc.sync.dma_start(out=miv[:], in_=is_missing[:].rearrange("(p a) -> p a", p=P))
            nc.vector.tensor_scalar_mul(out=miv[:], in0=miv[:], scalar1=100)
            nc.vector.tensor_tensor(out=cmv[:], in0=cmv[:], in1=miv[:], op=ALU.add)
            nc.sync.dma_start(out=CS8[:].rearrange("(p a) -> p a", p=P), in_=cmv[:])

        slab_cm = tc.tile_pool(name="sb_slab", bufs=1)
        slab_pool = slab_cm.__enter__()
        cs_slab = slab_pool.tile([P, SLAB], u8)
        dm_slab = slab_pool.tile([P, SLAB], u8)
        for g in range(NG):
            nc.sync.dma_start(
                out=cs_slab[16 * g:16 * (g + 1), :],
                in_=CS8[:].rearrange("(q s) -> q s", q=16))
            nc.sync.dma_start(
                out=dm_slab[16 * g:16 * (g + 1), :],
                in_=drug_map[:].rearrange("(q s) -> q s", q=16))

        # ======== phase 2+3: lookup pieces (emitted interleaved with chunks) ========
        g_cs = sb.tile([P, GS], u8)
        g_dm = sb.tile([P, GS], u8)
        ps_pool = [None]
        selsb_cm = tc.tile_pool(name="sb_sel", bufs=2)
        sb_sel = selsb_cm.__enter__()
        v8_cs = sb.tile([NG, GS], bf16, tag="v8_cs")
        v8_dm = sb.tile([NG, GS], bf16, tag="v8_dm")
        v8 = {"cs": v8_cs, "dm": v8_dm}

        def emit_piece(t):
            jsl = slice(t * 512, (t + 1) * 512)
            isl = slice(t * 32, (t + 1) * 32)
            for (name, gt, qt, ut, slab, rowd) in (
                    ("cs", g_cs, q_idx_sb, u_idx_sb, cs_slab, cs_rowd[t]),
                    ("dm", g_dm, q_tidx_sb, u_tidx_sb, dm_slab, dm_rowd[t])):
                nc.gpsimd.indirect_copy(
                    out=gt[:, jsl].rearrange("p (n one) -> p n one", one=1),
                    data=slab[:], idxs=ut[:, isl],
                    i_know_ap_gather_is_preferred=True)
                qmask = sb_sel.tile([P, 512], bf16, tag="qmask")
                qb = ps_pool[0].tile([P, 512], f32, tag="h1", name="qb")
                nc.tensor.matmul(out=qb[:], lhsT=grp_bc_sb[:], rhs=qt[:, jsl],
                                 start=True, stop=True)
                nc.vector.tensor_scalar(
                    out=qmask[:], in0=qb[:], scalar1=qi_sb[:], scalar2=None,
                    op0=ALU.is_equal)
                gf = sb_sel.tile([P, 512], bf16, tag="gf")
                nc.vector.tensor_copy(out=gf[:], in_=gt[:, jsl])
                nc.vector.tensor_tensor(out=gf[:], in0=gf[:], in1=qmask[:],
                                        op=ALU.mult)
                vpf = ps_pool[0].tile([P, 512], f32, tag="h1", name="vpf")
                nc.tensor.matmul(out=vpf[:NG, :], lhsT=grp_rd_sb[:], rhs=gf[:],
                                 start=True, stop=True)
                nc.vector.tensor_copy(out=v8[name][:, jsl], in_=vpf[:NG, :])
                nc.sync.dma_start(
                    out=rowd[:].rearrange("(g j) -> g j", g=NG),
                    in_=v8[name][:, jsl])

        # ======== params to SBUF (ACT HWDGE queue) ========
        w1_kt = []
        for kt in range(8):
            t = sbw.tile([P, CEMB], bf16, tag=f"w1_{kt}")
            nc.scalar.dma_start(out=t[:], in_=W1[kt * P:(kt + 1) * P, :])
            w1_kt.append(t)
        wf1c_kt = []
        for kt in range(8):
            t = sbw.tile([P, HID], bf16, tag=f"wf1c_{kt}")
            nc.scalar.dma_start(out=t[:], in_=Wf1c[kt * P:(kt + 1) * P, :])
            wf1c_kt.append(t)
        wf1d_sb = sbw.tile([DEMB, HID], bf16)
        nc.scalar.dma_start(out=wf1d_sb[:], in_=Wf1d[:])
        wf2_k0 = sbw.tile([P, HID], bf16)
        wf2_k1 = sbw.tile([HID - P, HID], bf16)
        nc.scalar.dma_start(out=wf2_k0[:], in_=Wf2[0:P, :])
        nc.scalar.dma_start(out=wf2_k1[:], in_=Wf2[P:HID, :])
        wf3a0_sb = sbw.tile([P, NDOSES - 1], bf16)
        wf3a1p_sb = sbw.tile([P, NDOSES - 1], bf16)
        nc.scalar.dma_start(out=wf3a0_sb[:], in_=Wf3a0[:])
        nc.scalar.dma_start(out=wf3a1p_sb[:], in_=Wf3a1p[:])
        mb0_sb = sbw.tile([P, NDOSES], bf16)
        mb1p_sb = sbw.tile([P, NDOSES], bf16)
        nc.scalar.dma_start(out=mb0_sb[:], in_=Mb0[:])
        nc.scalar.dma_start(out=mb1p_sb[:], in_=Mb1p[:])
        # static padded h2s m1 tile: rows 0:72 live, 73..126 zero, 127 ones
        h2s1_st = sbw.tile([P, 512], bf16)
        nc.vector.memset(h2s1_st[:], 0.0)
        nc.sync.dma_start(out=h2s1_st[P - 1:P, :], in_=onesrow[:])
        h2s0_st = sbw.tile([P, 512], bf16)
        b1_row = sbw.tile([1, CEMB], f32)
        nc.scalar.dma_start(out=b1_row[:], in_=b1[:].rearrange("(one n) -> one n", one=1))
        bf1_row = sbw.tile([1, HID], f32)
        nc.scalar.dma_start(out=bf1_row[:], in_=bf1[:].rearrange("(one n) -> one n", one=1))
        bf2_c0 = sbw.tile([P, 1], f32)
        bf2_c1 = sbw.tile([HID - P, 1], f32)
        nc.scalar.dma_start(out=bf2_c0[:], in_=bf2[0:P].rearrange("(p one) -> p one", one=1))
        nc.scalar.dma_start(out=bf2_c1[:], in_=bf2[P:HID].rearrange("(p one) -> p one", one=1))

        cft_kt = []
        for kt in range(8):
            t = sbw.tile([P, 100], bf16, tag=f"cft_{kt}")
            nc.scalar.dma_start(out=t[:], in_=cfT[kt * P:(kt + 1) * P, :])
            cft_kt.append(t)
        me_sb = sb.tile([100, CEMB], f32)
        nc.scalar.dma_start(out=me_sb[:], in_=me_in[:])
        de_p = []
        for mt in range(2):
            t = sb.tile([P, DEMB], f32, tag=f"de_{mt}")
            nc.scalar.dma_start(out=t[:], in_=drug_emb[mt * P:(mt + 1) * P, :])
            de_p.append(t)
        deT_sb = sb.tile([DEMB, NDRUG], bf16)
        nc.scalar.dma_start(out=deT_sb[:], in_=drug_embT[:])

        # ======== table construction: A [200,200], Bd [256,200] (bf16) ========
        with (
            tc.tile_pool(name="ps_tr", bufs=3, space="PSUM") as ps_tr,
            tc.tile_pool(name="ps_ps", bufs=2, space="PSUM") as ps_ps,
            tc.tile_pool(name="ps_aa", bufs=1, space="PSUM") as ps_aa,
        ):
            from concourse.masks import make_identity
            ident = sbw.tile([P, P], f32)
            make_identity(nc, ident[:])

            # P100 = relu(cf @ W1 + b1)  [100, 1024]
            p_sb = sb.tile([100, CEMB], f32)
            for nh in range(2):
                pps = ps_ps.tile([100, 512], f32, tag="pshard")
                for kt in range(8):
                    nc.tensor.matmul(
                        out=pps[:], lhsT=cft_kt[kt][:],
                        rhs=w1_kt[kt][:, nh * 512:(nh + 1) * 512],
                        start=(kt == 0), stop=False)
                nc.tensor.matmul(
                    out=pps[:], lhsT=ones_c100[:],
                    rhs=b1_row[:, nh * 512:(nh + 1) * 512], start=False, stop=True)
                nc.scalar.activation(
                    out=p_sb[:, nh * 512:(nh + 1) * 512], in_=pps[:], func=AF.Relu)

            # norm scales, scaled rows
            sq = sb.tile([100, CEMB], f32)
            ssp = sb.tile([100, 1], f32)
            ssm = sb.tile([100, 1], f32)
            nc.scalar.activation(out=sq[:], in_=p_sb[:], func=AF.Square)
            nc.vector.reduce_sum(out=ssp[:], in_=sq[:], axis=mybir.AxisListType.X)
            nc.scalar.activation(out=sq[:], in_=me_sb[:], func=AF.Square)
            nc.vector.reduce_sum(out=ssm[:], in_=sq[:], axis=mybir.AxisListType.X)
            for ss in (ssp, ssm):
                nc.scalar.activation(out=ss[:], in_=ss[:], func=AF.Sqrt)
                nc.vector.tensor_scalar_max(out=ss[:], in0=ss[:], scalar1=EPS)
                nc.vector.reciprocal(out=ss[:], in_=ss[:])
            nc.vector.tensor_scalar_mul(out=p_sb[:], in0=p_sb[:], scalar1=ssp[:])
            nc.vector.tensor_scalar_mul(out=me_sb[:], in0=me_sb[:], scalar1=ssm[:])
            cp_sb, cm_sb = p_sb, me_sb

            # CnT k-tiles [128, 200] bf16 (cols: 100 present + 100 missing)
            cnt_kt = []
            for kt in range(8):
                t = sb.tile([P, 2 * 100], bf16, tag=f"cnt_{kt}")
                for (src, co) in ((cp_sb, 0), (cm_sb, 100)):
                    tp = ps_tr.tile([P, 100], f32, tag="tr")
                    nc.tensor.transpose(
                        out=tp[:], in_=src[:, kt * P:(kt + 1) * P],
                        identity=ident[:100, :100])
                    nc.vector.tensor_copy(out=t[:, co:co + 100], in_=tp[:])
                cnt_kt.append(t)

            # A k-tiles (states on partitions): A0 [128, 200], A1 [72, 200]
            a_k = []
            for (mt, msl) in ((0, slice(0, P)), (1, slice(P, HID))):
                mm = msl.stop - msl.start
                aps = ps_aa.tile([P, HID], f32, tag="a")
                for kt in range(8):
                    nc.tensor.matmul(
                        out=aps[:mm, :], lhsT=cnt_kt[kt][:, msl],
                        rhs=wf1c_kt[kt][:], start=(kt == 0), stop=False)
                nc.tensor.matmul(
                    out=aps[:mm, :], lhsT=ones_c128[:, :mm], rhs=bf1_row[:],
                    start=False, stop=True)
                t = sb.tile([mm, HID], bf16, tag=f"a_{mt}")
                nc.vector.tensor_copy(out=t[:], in_=aps[:mm, :])
                a_k.append(t)

            # drug: rd scales + Bd k-tiles [128, 200] bf16 x2 (drugs on partitions)
            bd_k = []
            for mt in range(2):
                sqd = sb.tile([P, DEMB], f32, tag="sqd")
                rd = sb.tile([P, 1], f32, tag=f"rd_{mt}")
                nc.scalar.activation(out=sqd[:], in_=de_p[mt][:], func=AF.Square)
                nc.vector.reduce_sum(out=rd[:], in_=sqd[:], axis=mybir.AxisListType.X)
                nc.scalar.activation(out=rd[:], in_=rd[:], func=AF.Sqrt)
                nc.vector.tensor_scalar_max(out=rd[:], in0=rd[:], scalar1=EPS)
                nc.vector.reciprocal(out=rd[:], in_=rd[:])
                bps = ps_aa.tile([P, HID], f32, tag="bd")
                nc.tensor.matmul(out=bps[:], lhsT=deT_sb[:, mt * P:(mt + 1) * P],
                                 rhs=wf1d_sb[:], start=True, stop=True)
                t = sb.tile([P, HID], bf16, tag=f"bd_{mt}")
                nc.vector.tensor_scalar_mul(out=t[:], in0=bps[:], scalar1=rd[:])
                bd_k.append(t)

        # ======== per-sample chunk pipeline ========
        with (
            tc.tile_pool(name="ps_h1", bufs=2, space="PSUM") as ps_h1,
            tc.tile_pool(name="ps_h2", bufs=2, space="PSUM") as ps_h2,
            tc.tile_pool(name="ps_f9", bufs=2, space="PSUM") as ps_f9,
            tc.tile_pool(name="ps_mu", bufs=2, space="PSUM") as ps_mu,
            tc.tile_pool(name="sbc", bufs=2) as sbc,
        ):
            ps_pool[0] = ps_h1
            gb8 = sb.tile([NDOSES - 1, BS], bf16)  # softplus rows (dose1..8)
            mu9 = sb.tile([NDOSES, BS], f32)     # mu transposed; host untransposes
            h1s_of = {}
            f9_of = {}
            mu_of = {}

            def emit_A(ch):
                # one-hot build + h1 matmuls + h1 relus
                pc = ch % 2
                po = (ch // 2) * 512
                bc2 = sbc.tile([P, 1024], bf16, tag="bc2")
                nc.scalar.dma_start(out=bc2[:], in_=bass.AP(
                    tensor=cs_rowd[pc].ap().tensor, offset=po,
                    ap=[[0, P], [0, 2], [1, 512]]))
                sc2 = sbc.tile([P, 1024], bf16, tag="sc2")
                nc.vector.tensor_tensor(out=sc2[:], in0=bc2[:], in1=iota2_sb[:],
                                        op=ALU.is_equal)
                bd22 = sbc.tile([P, 1024], bf16, tag="bd22")
                nc.scalar.dma_start(out=bd22[:], in_=bass.AP(
                    tensor=dm_rowd[pc].ap().tensor, offset=po,
                    ap=[[0, P], [0, 2], [1, 512]]))
                sd2 = sbc.tile([P, 1024], bf16, tag="sd2")
                nc.vector.tensor_tensor(out=sd2[:], in0=bd22[:], in1=iota2_sb[:],
                                        op=ALU.is_equal)
                h1s = []
                for (mt, msl) in ((0, slice(0, P)), (1, slice(P, HID))):
                    mm = msl.stop - msl.start
                    hp = ps_h1.tile([P, 512], f32, tag="h1", name="hp1")
                    nc.tensor.matmul(out=hp[:mm, :], lhsT=a_k[0][:, msl],
                                     rhs=sc2[:, 0:512], start=True, stop=False)
                    nc.tensor.matmul(out=hp[:mm, :], lhsT=a_k[1][:, msl],
                                     rhs=sc2[:HID - P, 512:1024], start=False, stop=False)
                    nc.tensor.matmul(out=hp[:mm, :], lhsT=bd_k[0][:, msl],
                                     rhs=sd2[:, 0:512], start=False, stop=False)
                    nc.tensor.matmul(out=hp[:mm, :], lhsT=bd_k[1][:, msl],
                                     rhs=sd2[:, 512:1024], start=False, stop=True)
                    hs = sbc.tile([mm, 512], bf16, tag=f"h1s_{mt}")
                    nc.vector.tensor_scalar_max(out=hs[:], in0=hp[:mm, :],
                                                scalar1=0.0)
                    h1s.append(hs)
                h1s_of[ch] = h1s

            def emit_B(ch):
                # h2 matmuls+relu, f9a, mu base part
                h1s = h1s_of.pop(ch)
                for (mt, msl, bfc, hout) in (
                        (0, slice(0, P), bf2_c0, h2s0_st),
                        (1, slice(P, HID), bf2_c1, h2s1_st)):
                    mm = msl.stop - msl.start
                    hp = ps_h2.tile([P, 512], f32, tag="h2")
                    nc.tensor.matmul(out=hp[:mm, :], lhsT=wf2_k0[:, msl], rhs=h1s[0][:],
                                     start=True, stop=False)
                    nc.tensor.matmul(out=hp[:mm, :], lhsT=wf2_k1[:, msl], rhs=h1s[1][:],
                                     start=False, stop=True)
                    nc.scalar.activation(out=hout[0:mm, :], in_=hp[:mm, :],
                                         func=AF.Relu, bias=bfc[:], scale=1.0)
                # f9a: softplus-dose raw activations (bias via ones row 127)
                f9 = ps_f9.tile([NDOSES - 1, 512], f32, tag="f9")
                nc.tensor.matmul(out=f9[:], lhsT=wf3a0_sb[:], rhs=h2s0_st[:],
                                 start=True, stop=False)
                nc.tensor.matmul(out=f9[:], lhsT=wf3a1p_sb[:], rhs=h2s1_st[:],
                                 start=False, stop=True)
                f9_of[ch] = f9
                # mu base: Mb^T h2 (+ bf3[dose0] via ones row); L8 part in tail
                mups = ps_mu.tile([NDOSES, 512], f32, tag="mu")
                nc.tensor.matmul(out=mups[:], lhsT=mb0_sb[:], rhs=h2s0_st[:],
                                 start=True, stop=False)
                nc.tensor.matmul(out=mups[:], lhsT=mb1p_sb[:], rhs=h2s1_st[:],
                                 start=False, stop=False)
                mu_of[ch] = mups

            def emit_tail(ch):
                n0 = ch * 512
                sl = slice(n0, n0 + 512)
                f9 = f9_of.pop(ch)
                mups = mu_of.pop(ch)
                nc.scalar.activation(out=gb8[:, sl], in_=f9[:], func=AF.Exp)
                nc.scalar.activation(out=gb8[:, sl], in_=gb8[:, sl],
                                     func=AF.Ln, bias=1.0, scale=1.0)
                nc.tensor.matmul(out=mups[:], lhsT=L8_sb[:], rhs=gb8[:, sl],
                                 start=False, stop=True)
                nc.vector.tensor_copy(out=mu9[:, sl], in_=mups[:])

            emit_piece(0)
            seq = list(range(0, NCHUNK, 2)) + list(range(1, NCHUNK, 2))
            for ci, ch in enumerate(seq):
                emit_A(ch)
                if ci == 0:
                    emit_piece(1)
                if ci >= 1:
                    emit_B(seq[ci - 1])
                if ci >= 2:
                    emit_tail(seq[ci - 2])
            emit_B(seq[-1])
            emit_tail(seq[-2])
            emit_tail(seq[-1])

            nc.sync.dma_start(out=mu9_s[:], in_=mu9[:])
        selsb_cm.__exit__(None, None, None)
        slab_cm.__exit__(None, None, None)

    return _split_sync_waits(nc) if split_waits else nc


def _get_nc():
    if "nc" not in _NC_CACHE:
        _NC_CACHE["nc"] = build_nc()
    return _NC_CACHE["nc"]



def _wf3p(inputs):
    return np.asarray(inputs["Wf3"], np.float32)[:, [1, 2, 3, 4, 5, 6, 7, 8, 0]]


def _bf3p(inputs):
    return np.asarray(inputs["bf3"], np.float32)[[1, 2, 3, 4, 5, 6, 7, 8, 0]]


def _wf3a0(inputs):
    return np.ascontiguousarray(_wf3p(inputs)[0:P, 0:NDOSES - 1].astype(np_bf16))


def _wf3a1p(inputs):
    m = np.zeros((P, NDOSES - 1), np.float32)
    m[0:HID - P] = _wf3p(inputs)[P:HID, 0:NDOSES - 1]
    m[P - 1] = _bf3p(inputs)[0:NDOSES - 1]
    return np.ascontiguousarray(m.astype(np_bf16))


def _mb0(inputs):
    return np.ascontiguousarray(
        np.tile(_wf3p(inputs)[0:P, NDOSES - 1:NDOSES], (1, NDOSES)).astype(np_bf16))


def _mb1p(inputs):
    m = np.zeros((P, NDOSES), np.float32)
    m[0:HID - P] = np.tile(_wf3p(inputs)[P:HID, NDOSES - 1:NDOSES], (1, NDOSES))
    m[P - 1] = _bf3p(inputs)[NDOSES - 1]
    return np.ascontiguousarray(m.astype(np_bf16))


def make_in_maps(inputs):
    idx = np.asarray(inputs["idx"], np.int64)
    tidx = np.asarray(inputs["tidx"], np.int64)
    cf = np.asarray(inputs["cell_features"], np.float32)
    me = np.asarray(inputs["missing_emb"], np.float32)
    de = np.asarray(inputs["drug_emb"], np.float32)
    Wf1 = np.asarray(inputs["Wf1"], np.float32)

    shared = dict(
        cell_map=np.ascontiguousarray(np.asarray(inputs["cell_map"]).astype(np.uint8)),
        is_missing=np.ascontiguousarray(np.asarray(inputs["is_missing"]).astype(np.uint8)),
        drug_map=np.ascontiguousarray(np.asarray(inputs["drug_map"]).astype(np.uint8)),
        cfT=np.ascontiguousarray(cf[:100, :].T.astype(np_bf16)),
        me_in=np.ascontiguousarray(me),
        drug_emb=np.ascontiguousarray(de),
        drug_embT=np.ascontiguousarray(de.T.astype(np_bf16)),
        W1=np.ascontiguousarray(np.asarray(inputs["W1"], np.float32).astype(np_bf16)),
        b1=np.ascontiguousarray(np.asarray(inputs["b1"], np.float32)),
        Wf1c=np.ascontiguousarray(Wf1[:CEMB, :].astype(np_bf16)),
        Wf1d=np.ascontiguousarray(Wf1[CEMB:, :].astype(np_bf16)),
        bf1=np.ascontiguousarray(np.asarray(inputs["bf1"], np.float32)),
        Wf2=np.ascontiguousarray(np.asarray(inputs["Wf2"], np.float32).astype(np_bf16)),
        bf2=np.ascontiguousarray(np.asarray(inputs["bf2"], np.float32)),
        Wf3a0=_wf3a0(inputs), Wf3a1p=_wf3a1p(inputs),
        Mb0=_mb0(inputs), Mb1p=_mb1p(inputs),
        onesrow=np.ascontiguousarray(np.ones((1, 512), np_bf16)),
    )

    def wrap16(vals):
        # vals [8192] in sample order k (g = k>>10, j = k&1023)
        # -> [128, 64] at [16g + (j & 15), j >> 4]
        v = vals.reshape(NG, GS // 16, 16)        # [g, j_hi, j_lo]
        v = np.transpose(v, (0, 2, 1))            # [g, j_lo, j_hi]
        return np.ascontiguousarray(v.reshape(P, GS // 16))

    in_maps = []
    for c in range(NCORES):
        ic = idx[c * BS:(c + 1) * BS]
        tc_ = tidx[c * BS:(c + 1) * BS]
        m = dict(shared)
        m["u_idx"] = wrap16((ic & (SLAB - 1)).astype(np.uint16))
        m["u_tidx"] = wrap16((tc_ & (SLAB - 1)).astype(np.uint16))
        m["q_idx"] = np.ascontiguousarray(
            (ic >> 14).astype(np_bf16).reshape(NG, GS))
        m["q_tidx"] = np.ascontiguousarray(
            (tc_ >> 14).astype(np_bf16).reshape(NG, GS))
        in_maps.append(m)
    return in_maps


def kernel(**inputs):
    nc = _get_nc()
    in_maps = make_in_maps(inputs)
    last_err = None
    for _attempt in range(3):
        try:
            res = run_bass_kernel_spmd(nc, in_maps, core_ids=list(range(NCORES)))
            return np.concatenate(
                [np.ascontiguousarray(res.results[c]["mu9_s"].T)
                 for c in range(NCORES)], axis=0)
        except Exception as e:  # wedged device sometimes recovers on retry
            last_err = e
    raise last_err



# revision 33
# speedup vs baseline: 1.2036x; 1.2036x over previous
"""Trainium2 Bass kernel for nn_DrugResponsePrior (embedding_lookup).

Spec guarantees: cell_map < 100, is_missing in {0,1}, drug_map < 256.  Each
row's result depends only on the cell state cs = cell_map[idx] +
100*is_missing[idx] (200 states) and dm = drug_map[tidx] (256 drugs).

Fully data-parallel (8 cores x 8192 samples, no collectives).  Per core:
  1. Host bit-packs csmi = cell_map | (is_missing << 7) (u8 - a pure bit
     repack; the state CODE cm + 128*mi is exact in bf16).  csmi/drug_map are
     loaded in a 16-slab SBUF layout (partition 16g+r holds entries
     [r*16384, (r+1)*16384) for every group g).
  2. Two GPSIMD indirect_copy gathers per 4096-sample piece fetch the 16
     slab candidates per sample; a one-hot mask over idx>>14 (grp_bc matmul +
     is_eq) and a group-reduce matmul produce v8 = per-sample code [8, 1024].
  3. Tables built once on device: A = l2n(cell emb) @ Wf1c + bf1 ([200,200])
     and Bd = l2n(drug_emb) @ Wf1d ([256,200]), bf16.
  4. Per 512-sample chunk: gpsimd partition_broadcast expands the codes to
     [128, 512]; two DVE is_eq (4x mode, bf16) build one-hot matrices; bf16
     matmuls run the MLP: h1 = relu(A^T Sc + Bd^T Sd), h2 = relu(Wf2^T h1 +
     bf2), fm = [fwd(1:9); mu-base] via one packed [.,17] lhsT, softplus on
     scalar engine, one L8 matmul accumulates the cumsum into the mu rows.
  Chunks are software-pipelined depth 3 so every engine streams without
  gaps (keeps the PE p-state at full clock).

All params ride in two packed blob tensors ([128, N] with large contiguous
partition lines) so the whole param load is 2 DMAs - the baseline's ~16k
small DMA descriptors were the main bottleneck.

All reference math runs on device; the host only reshapes/transposes/casts
inputs, bit-packs the two sub-byte index tables, and slices idx/tidx (pure
index arithmetic: & 16383, >> 14).
"""
import sys

if "/opt/trn_rl_repo" not in sys.path:
    sys.path.insert(0, "/opt/trn_rl_repo")

import numpy as np
import ml_dtypes

import concourse.bass as bass
import concourse.bass_isa as bass_isa
import concourse.mybir as mybir
import concourse.tile as tile
from concourse.bass_utils import run_bass_kernel_spmd

f32 = mybir.dt.float32
bf16 = mybir.dt.bfloat16
u16 = mybir.dt.uint16
u8 = mybir.dt.uint8
np_bf16 = ml_dtypes.bfloat16

B = 65536
R = 262144
NDRUG = 256
NFEAT = 1024
CEMB = 1024
DEMB = 128
HID = 200
NDOSES = 9
NCORES = 8

BS = B // NCORES            # 8192 samples per core
P = 128
NG = 8                      # gpsimd groups (16 partitions each)
GS = BS // NG               # 1024 samples per group
SLAB = R // 16              # 16384 entries per slab partition
EPS = 1e-12

_NC_CACHE = {}

# ---------------- packed blob layouts (host & device share these) ----------
def _layout(specs):
    out, off = {}, 0
    for name, n in specs:
        out[name] = (off, off + n)
        off += n
    return out, off

BF_L, NBF = _layout([
    ("w1", 8 * CEMB),        # 8 k-tiles [128, 1024]
    ("wf1c", 8 * HID),       # 8 k-tiles [128, 200]
    ("cft", 8 * 100),        # 8 k-tiles [128, 100]
    ("wf1d", HID),           # [128, 200]
    ("deT", NDRUG),          # [128, 256] drug_emb^T
    ("wf2a", HID),           # Wf2[0:128, :]
    ("wf2b", HID),           # rows 0:72 = Wf2[128:200, :]
    ("fma", 17),             # [Wf3p[0:128, 0:8] | tile(col base, 9)]
    ("fmb", 17),             # rows 0:72 = Wf3p[128:200]; row 127 = biases
    ("l8", 17),              # rows 0:8: cols 0:8 zero, cols 8:17 (k < o)
    ("grp_bc", P),           # rows 0:8: [g, p] = (p//16 == g)
    ("grp_rd", NG),          # [p, g] = (p//16 == g)
    ("selg", NG * P),        # block g: [g', p] = (g' == g)  (bcast lhsT)
    ("ones512", 512),        # row 0 = 1.0 (DMA'd to h2s1_st row 127)
])

F_L, NF32 = _layout([
    ("meb", CEMB),           # rows 0:100 = missing_emb
    ("b1r", CEMB),           # row 0 = b1 (matmul rhs: base partition must be 0)
    ("bf1r", HID),           # row 0 = bf1
    ("onesr", P),            # row 0 = ones
    ("de", 2 * DEMB),        # drug_emb [256, 128] as two [128, 128] tiles
    ("qi", 1),               # p % 16
    ("ccl", 1),              # cs code, one-hot block lo
    ("cch", 1),              # cs code, block hi
    ("cdl", 1),              # dm code lo
    ("cdh", 1),              # dm code hi
    ("bf2a", 1),
    ("bf2b", 1),
])


def _split_sync_waits(nc, limit=1):
    """The walrus accepts at most one sync-wait per instruction; hoist excess
    waits onto same-engine NoOps inserted just before."""
    ctr = 0
    for bb in nc.main_func.blocks:
        new_list = []
        for inst in bb.instructions:
            si = inst.sync_info
            if si is not None and si.on_wait and len(si.on_wait) > limit:
                waits = list(si.on_wait)
                head, tail = waits[:-limit], waits[-limit:]
                for j in range(0, len(head), limit):
                    nop = mybir.InstNoOp(name=f"waitnop-{ctr}", engine=inst.engine)
                    ctr += 1
                    nop.sync_info = mybir.SyncInfo(
                        on_wait=list(head[j : j + limit]), on_update=[]
                    )
                    new_list.append(nop)
                inst.sync_info = mybir.SyncInfo(
                    on_wait=list(tail),
                    on_update=list(si.on_update) if si.on_update else [],
                )
            new_list.append(inst)
        bb.instructions[:] = new_list
    return nc


def build_nc(split_waits=True):
    nc = bass.Bass(num_devices=NCORES)
    AF = mybir.ActivationFunctionType
    ALU = mybir.AluOpType

    # ---------------- kernel I/O ----------------
    blob_bf = nc.dram_tensor("blob_bf", [P, NBF], bf16, kind="ExternalInput")
    blob_f32 = nc.dram_tensor("blob_f32", [P, NF32], f32, kind="ExternalInput")
    csmi = nc.dram_tensor("csmi", [R], u8, kind="ExternalInput")
    dm_map = nc.dram_tensor("dm_map", [R], u8, kind="ExternalInput")
    u_idx = nc.dram_tensor("u_idx", [P, GS // 16], u16, kind="ExternalInput")
    u_tidx = nc.dram_tensor("u_tidx", [P, GS // 16], u16, kind="ExternalInput")
    q_idx = nc.dram_tensor("q_idx", [NG, GS], bf16, kind="ExternalInput")
    q_tidx = nc.dram_tensor("q_tidx", [NG, GS], bf16, kind="ExternalInput")
    mu9_s = nc.dram_tensor("mu9_s", [NDOSES, BS], f32, kind="ExternalOutput")

    def bsl(t, name, rows=None):
        lo, hi = BF_L[name] if t is bb_sb else F_L[name]
        if rows is None:
            return t[:, lo:hi]
        return t[rows[0]:rows[1], lo:hi]

    with tile.TileContext(nc) as tc, \
            tc.tile_pool(name="sbw", bufs=1) as sbw, \
            tc.tile_pool(name="sb", bufs=1) as sb:

        # ---- setup DMAs ----
        u_idx_sb = sb.tile([P, GS // 16], u16)
        u_tidx_sb = sb.tile([P, GS // 16], u16)
        q_idx_sb = sb.tile([NG, GS], bf16)
        q_tidx_sb = sb.tile([NG, GS], bf16)
        nc.sync.dma_start(out=u_idx_sb[:], in_=u_idx[:])
        nc.sync.dma_start(out=u_tidx_sb[:], in_=u_tidx[:])
        nc.sync.dma_start(out=q_idx_sb[:], in_=q_idx[:])
        nc.sync.dma_start(out=q_tidx_sb[:], in_=q_tidx[:])

        bb_sb = sbw.tile([P, NBF], bf16)
        bf_sb = sbw.tile([P, NF32], f32)
        nc.scalar.dma_start(out=bb_sb[:], in_=blob_bf[:])
        nc.scalar.dma_start(out=bf_sb[:], in_=blob_f32[:])

        cs_slab = sbw.tile([P, SLAB], u8)
        dm_slab = sbw.tile([P, SLAB], u8)
        nc.sync.dma_start(out=cs_slab[:], in_=bass.AP(
            tensor=csmi.ap().tensor, offset=0,
            ap=[[0, NG], [SLAB, 16], [1, SLAB]]))
        nc.sync.dma_start(out=dm_slab[:], in_=bass.AP(
            tensor=dm_map.ap().tensor, offset=0,
            ap=[[0, NG], [SLAB, 16], [1, SLAB]]))

        # blob views
        me_sb = bsl(bf_sb, "meb", (0, 100))
        b1_row = bsl(bf_sb, "b1r", (0, 1))
        bf1_row = bsl(bf_sb, "bf1r", (0, 1))
        ones100 = bf_sb[0:1, F_L["onesr"][0]:F_L["onesr"][0] + 100]
        ones128 = bsl(bf_sb, "onesr", (0, 1))
        de0 = bf_sb[:, F_L["de"][0]:F_L["de"][0] + DEMB]
        de1 = bf_sb[:, F_L["de"][0] + DEMB:F_L["de"][0] + 2 * DEMB]
        qi_c = bsl(bf_sb, "qi")
        ccl_c = bsl(bf_sb, "ccl")
        cch_c = bsl(bf_sb, "cch")
        cdl_c = bsl(bf_sb, "cdl")
        cdh_c = bsl(bf_sb, "cdh")
        bf2a_c = bsl(bf_sb, "bf2a")
        bf2b_c = bf_sb[0:72, F_L["bf2b"][0]:F_L["bf2b"][1]]
        w1_kt = [bb_sb[:, BF_L["w1"][0] + k * CEMB:BF_L["w1"][0] + (k + 1) * CEMB]
                 for k in range(8)]
        wf1c_kt = [bb_sb[:, BF_L["wf1c"][0] + k * HID:BF_L["wf1c"][0] + (k + 1) * HID]
                   for k in range(8)]
        cft_kt = [bb_sb[:, BF_L["cft"][0] + k * 100:BF_L["cft"][0] + (k + 1) * 100]
                  for k in range(8)]
        wf1d_sb = bsl(bb_sb, "wf1d")
        deT_sb = bsl(bb_sb, "deT")
        wf2a = bsl(bb_sb, "wf2a")
        wf2b = bsl(bb_sb, "wf2b", (0, 72))
        fma = bsl(bb_sb, "fma")
        fmb = bsl(bb_sb, "fmb")
        l8_sb = bsl(bb_sb, "l8", (0, 8))
        grp_bc = bsl(bb_sb, "grp_bc", (0, NG))
        grp_rd = bsl(bb_sb, "grp_rd")
        selg = [bb_sb[0:NG, BF_L["selg"][0] + g * P:BF_L["selg"][0] + (g + 1) * P]
                for g in range(NG)]

        # static h2 tiles (relu outputs; h2s1 rows 72:127 zero, row 127 ones
        # so fmb's row 127 supplies the biases)
        h2s0_st = sbw.tile([P, 512], bf16)
        h2s1_st = sbw.tile([P, 512], bf16)
        nc.vector.memset(h2s1_st[:], 0.0)
        nc.sync.dma_start(
            out=h2s1_st[P - 1:P, :],
            in_=blob_bf[0:1, BF_L["ones512"][0]:BF_L["ones512"][1]])

        # ======== table construction: A [200,200], Bd [256,200] (bf16) ======
        a_k = []
        bd_k = []
        with (
            tc.tile_pool(name="ps_tb", bufs=1, space="PSUM") as ps_tb,
            tc.tile_pool(name="ps_tr", bufs=3, space="PSUM") as ps_tr,
            tc.tile_pool(name="sbt", bufs=1) as sbt,
        ):
            from concourse.masks import make_identity
            ident = sbt.tile([P, P], f32)
            make_identity(nc, ident[:])

            # P100 = relu(cf @ W1 + b1)  [100, 1024]
            p_sb = sbt.tile([100, CEMB], f32)
            for nh in range(2):
                pps = ps_tb.tile([100, 512], f32, tag="pshard")
                for kt in range(8):
                    nc.tensor.matmul(
                        out=pps[:], lhsT=cft_kt[kt],
                        rhs=w1_kt[kt][:, nh * 512:(nh + 1) * 512],
                        start=(kt == 0), stop=False)
                nc.tensor.matmul(
                    out=pps[:], lhsT=ones100,
                    rhs=b1_row[:, nh * 512:(nh + 1) * 512], start=False, stop=True)
                nc.scalar.activation(
                    out=p_sb[:, nh * 512:(nh + 1) * 512], in_=pps[:], func=AF.Relu)

            # l2 norm scales for present / missing rows
            sq = sbt.tile([100, CEMB], f32)
            ssp = sbt.tile([100, 1], f32)
            ssm = sbt.tile([100, 1], f32)
            nc.scalar.activation(out=sq[:], in_=p_sb[:], func=AF.Square)
            nc.vector.reduce_sum(out=ssp[:], in_=sq[:], axis=mybir.AxisListType.X)
            nc.scalar.activation(out=sq[:], in_=me_sb, func=AF.Square)
            nc.vector.reduce_sum(out=ssm[:], in_=sq[:], axis=mybir.AxisListType.X)
            for ss in (ssp, ssm):
                nc.scalar.activation(out=ss[:], in_=ss[:], func=AF.Sqrt)
                nc.vector.tensor_scalar_max(out=ss[:], in0=ss[:], scalar1=EPS)
                nc.vector.reciprocal(out=ss[:], in_=ss[:])
            nc.vector.tensor_scalar_mul(out=p_sb[:], in0=p_sb[:], scalar1=ssp[:])
            nc.vector.tensor_scalar_mul(out=me_sb, in0=me_sb, scalar1=ssm[:])

            # CnT k-tiles [128, 200] bf16 (cols: 100 present + 100 missing)
            cnt_kt = []
            for kt in range(8):
                t = sbt.tile([P, 2 * 100], bf16, tag=f"cnt_{kt}")
                for (src, co) in ((p_sb[:], 0), (me_sb, 100)):
                    tp = ps_tr.tile([P, 100], f32, tag="tr")
                    nc.tensor.transpose(
                        out=tp[:], in_=src[:, kt * P:(kt + 1) * P],
                        identity=ident[:100, :100])
                    nc.vector.tensor_copy(out=t[:, co:co + 100], in_=tp[:])
                cnt_kt.append(t)

            # A tiles (states on partitions): a_k[0] [128, 200], a_k[1] [72, 200]
            for (mt, msl) in ((0, slice(0, P)), (1, slice(P, HID))):
                mm = msl.stop - msl.start
                aps = ps_tb.tile([P, HID], f32, tag="a")
                for kt in range(8):
                    nc.tensor.matmul(
                        out=aps[:mm, :], lhsT=cnt_kt[kt][:, msl],
                        rhs=wf1c_kt[kt], start=(kt == 0), stop=False)
                nc.tensor.matmul(
                    out=aps[:mm, :], lhsT=ones128[:, :mm], rhs=bf1_row,
                    start=False, stop=True)
                t = sb.tile([mm, HID], bf16, tag=f"a_{mt}")
                nc.vector.tensor_copy(out=t[:], in_=aps[:mm, :])
                a_k.append(t)

            # drug tiles: per-drug l2 recip + Bd [128, 200] bf16 x2
            for (mt, de_p) in ((0, de0), (1, de1)):
                sqd = sbt.tile([P, DEMB], f32, tag="sqd")
                rd = sbt.tile([P, 1], f32, tag=f"rd_{mt}")
                nc.scalar.activation(out=sqd[:], in_=de_p, func=AF.Square)
                nc.vector.reduce_sum(out=rd[:], in_=sqd[:], axis=mybir.AxisListType.X)
                nc.scalar.activation(out=rd[:], in_=rd[:], func=AF.Sqrt)
                nc.vector.tensor_scalar_max(out=rd[:], in0=rd[:], scalar1=EPS)
                nc.vector.reciprocal(out=rd[:], in_=rd[:])
                bps = ps_tb.tile([P, HID], f32, tag="a")
                nc.tensor.matmul(out=bps[:], lhsT=deT_sb[:, mt * P:(mt + 1) * P],
                                 rhs=wf1d_sb, start=True, stop=True)
                t = sb.tile([P, HID], bf16, tag=f"bd_{mt}")
                nc.vector.tensor_scalar_mul(out=t[:], in0=bps[:], scalar1=rd[:])
                bd_k.append(t)

            # ======== lookup pieces (both up front) ========
            g_cs = sbt.tile([P, GS], u8)
            g_dm = sbt.tile([P, GS], u8)
            v8_cs = sb.tile([NG, GS], bf16)
            v8_dm = sb.tile([NG, GS], bf16)
            for t in range(2):
                nc.gpsimd.indirect_copy(
                    out=g_cs[:, t * 512:(t + 1) * 512].rearrange(
                        "p (n one) -> p n one", one=1),
                    data=cs_slab[:], idxs=u_idx_sb[:, t * 32:(t + 1) * 32],
                    i_know_ap_gather_is_preferred=True)
                nc.gpsimd.indirect_copy(
                    out=g_dm[:, t * 512:(t + 1) * 512].rearrange(
                        "p (n one) -> p n one", one=1),
                    data=dm_slab[:], idxs=u_tidx_sb[:, t * 32:(t + 1) * 32],
                    i_know_ap_gather_is_preferred=True)
            for t in range(2):
                jsl = slice(t * 512, (t + 1) * 512)
                for (gt, qt, v8t) in ((g_cs, q_idx_sb, v8_cs),
                                      (g_dm, q_tidx_sb, v8_dm)):
                    qb = ps_tb.tile([P, 512], f32, tag="pqb")
                    nc.tensor.matmul(out=qb[:], lhsT=grp_bc, rhs=qt[:, jsl],
                                     start=True, stop=True)
                    qmask = sbt.tile([P, 512], bf16, tag="qmask")
                    nc.vector.tensor_scalar(
                        out=qmask[:], in0=qb[:], scalar1=qi_c, scalar2=None,
                        op0=ALU.is_equal)
                    gf = sbt.tile([P, 512], bf16, tag="gf")
                    nc.vector.tensor_copy(out=gf[:], in_=gt[:, jsl])
                    nc.vector.tensor_tensor(out=gf[:], in0=gf[:], in1=qmask[:],
                                            op=ALU.mult)
                    vpf = ps_tb.tile([NG, 512], f32, tag="pvpf")
                    nc.tensor.matmul(out=vpf[:], lhsT=grp_rd, rhs=gf[:],
                                     start=True, stop=True)
                    nc.vector.tensor_copy(out=v8t[:, jsl], in_=vpf[:])

        # ======== per-chunk pipeline ========
        chunks = [(g, pc) for pc in range(2) for g in range(NG)]
        NCH = len(chunks)
        # rows 0:8 = spent f9 junk (not stored); rows 8:17 = mu
        mu9 = sb.tile([8 + NDOSES, BS], f32)

        with (
            tc.tile_pool(name="ps_h1", bufs=1, space="PSUM") as ps_h1,
            tc.tile_pool(name="ps_h2", bufs=1, space="PSUM") as ps_h2,
            tc.tile_pool(name="ps_fm", bufs=2, space="PSUM") as ps_fm,
            tc.tile_pool(name="ps_qb", bufs=1, space="PSUM") as ps_qb,
            tc.tile_pool(name="sbc", bufs=2) as sbc,
        ):
            bc_of, oh_of, h1ps_of, h1s_of, h2ps_of, fm_of, spb_of = \
                {}, {}, {}, {}, {}, {}, {}

            def emit_qb(i):
                # broadcast codes of chunk i's group to all 128 partitions
                g, pc = chunks[i]
                jsl = slice(pc * 512, (pc + 1) * 512)
                qbc = ps_qb.tile([P, 512], f32, tag="qbc")
                qbd = ps_qb.tile([P, 512], f32, tag="qbd")
                nc.tensor.matmul(out=qbc[:], lhsT=selg[g], rhs=v8_cs[:, jsl],
                                 start=True, stop=True)
                nc.tensor.matmul(out=qbd[:], lhsT=selg[g], rhs=v8_dm[:, jsl],
                                 start=True, stop=True)
                bc_of[i] = (qbc, qbd)

            def emit_bcopy(i):
                # bf16 SBUF copies so the is_eq runs in the DVE 4x mode;
                # cs on scalar, dm on vector to balance engine load
                qbc, qbd = bc_of.pop(i)
                bcc = sbc.tile([P, 512], bf16, tag="bcc")
                bcd = sbc.tile([P, 512], bf16, tag="bcd")
                nc.scalar.activation(out=bcc[:], in_=qbc[:], func=AF.Copy)
                nc.vector.tensor_copy(out=bcd[:], in_=qbd[:])
                bc_of[i] = (bcc, bcd)

            def emit_onehot(i):
                bcc, bcd = bc_of.pop(i)
                sc2 = sbc.tile([P, 1024], bf16, tag="sc2")
                sd2 = sbc.tile([P, 1024], bf16, tag="sd2")
                for (oh, bc, cl, ch_) in ((sc2, bcc, ccl_c, cch_c),
                                          (sd2, bcd, cdl_c, cdh_c)):
                    nc.vector.tensor_scalar(
                        out=oh[:, 0:512], in0=bc[:], scalar1=cl, scalar2=None,
                        op0=ALU.is_equal)
                    nc.vector.tensor_scalar(
                        out=oh[:, 512:1024], in0=bc[:], scalar1=ch_, scalar2=None,
                        op0=ALU.is_equal)
                oh_of[i] = (sc2, sd2)

            def emit_h1(i):
                sc2, sd2 = oh_of.pop(i)
                hps = []
                for (mt, msl) in ((0, slice(0, P)), (1, slice(P, HID))):
                    mm = msl.stop - msl.start
                    hp = ps_h1.tile([mm, 512], f32, tag=f"h1_{mt}")
                    nc.tensor.matmul(out=hp[:], lhsT=a_k[0][:, msl],
                                     rhs=sc2[:, 0:512], start=True, stop=False)
                    nc.tensor.matmul(out=hp[:], lhsT=a_k[1][:, msl],
                                     rhs=sc2[0:HID - P, 512:1024],
                                     start=False, stop=False)
                    nc.tensor.matmul(out=hp[:], lhsT=bd_k[0][:, msl],
                                     rhs=sd2[:, 0:512], start=False, stop=False)
                    nc.tensor.matmul(out=hp[:], lhsT=bd_k[1][:, msl],
                                     rhs=sd2[:, 512:1024], start=False, stop=True)
                    hps.append(hp)
                h1ps_of[i] = hps

            def emit_h1relu(i):
                hps = h1ps_of.pop(i)
                h1s = []
                for mt, hp in enumerate(hps):
                    mm = P if mt == 0 else HID - P
                    hs = sbc.tile([mm, 512], bf16, tag=f"h1s_{mt}")
                    nc.vector.tensor_scalar_max(out=hs[:], in0=hp[:], scalar1=0.0)
                    h1s.append(hs)
                h1s_of[i] = h1s

            def emit_h2(i):
                h1s = h1s_of.pop(i)
                hps = []
                for (mt, msl) in ((0, slice(0, P)), (1, slice(P, HID))):
                    mm = msl.stop - msl.start
                    hp = ps_h2.tile([mm, 512], f32, tag=f"h2_{mt}")
                    nc.tensor.matmul(out=hp[:], lhsT=wf2a[:, msl], rhs=h1s[0][:],
                                     start=True, stop=False)
                    nc.tensor.matmul(out=hp[:], lhsT=wf2b[:, msl], rhs=h1s[1][:],
                                     start=False, stop=True)
                    hps.append(hp)
                h2ps_of[i] = hps

            def emit_h2relu(i):
                hps = h2ps_of.pop(i)
                nc.scalar.activation(out=h2s0_st[:], in_=hps[0][:], func=AF.Relu,
                                     bias=bf2a_c, scale=1.0)
                nc.scalar.activation(out=h2s1_st[0:HID - P, :], in_=hps[1][:],
                                     func=AF.Relu, bias=bf2b_c, scale=1.0)

            def emit_fm(i):
                fm = ps_fm.tile([8 + NDOSES, 512], f32, tag="fm")
                nc.tensor.matmul(out=fm[:], lhsT=fma, rhs=h2s0_st[:],
                                 start=True, stop=False)
                nc.tensor.matmul(out=fm[:], lhsT=fmb, rhs=h2s1_st[:],
                                 start=False, stop=True)
                fm_of[i] = fm

            def emit_softplus(i):
                # softplus = ln(1 + exp(x)); Softplus isn't in CoreSim
                fm = fm_of[i]
                spb = sbc.tile([8, 512], bf16, tag="spb")
                nc.scalar.activation(out=spb[:], in_=fm[0:8, :], func=AF.Exp)
                nc.scalar.activation(out=spb[:], in_=spb[:], func=AF.Ln,
                                     bias=1.0, scale=1.0)
                spb_of[i] = spb

            def emit_l8(i):
                fm = fm_of[i]
                spb = spb_of.pop(i)
                nc.tensor.matmul(out=fm[:], lhsT=l8_sb, rhs=spb[:],
                                 start=False, stop=True, skip_group_check=True)

            def emit_mucopy(i):
                g, pc = chunks[i]
                fm = fm_of.pop(i)
                n0 = g * GS + pc * 512
                nc.vector.tensor_copy(out=mu9[:, n0:n0 + 512], in_=fm[:])

            # prologue
            emit_qb(0)
            emit_bcopy(0)
            emit_onehot(0)

            for i in range(NCH):
                emit_h1(i)
                emit_h1relu(i)
                if i + 1 < NCH:
                    emit_qb(i + 1)
                    emit_bcopy(i + 1)
                    emit_onehot(i + 1)
                if i >= 1:
                    emit_h2(i - 1)
                    emit_h2relu(i - 1)
                    emit_fm(i - 1)
                    emit_softplus(i - 1)
                if i >= 2:
                    emit_l8(i - 2)
                    emit_mucopy(i - 2)
            for i in (NCH - 1,):
                emit_h2(i)
                emit_h2relu(i)
                emit_fm(i)
                emit_softplus(i)
            emit_l8(NCH - 2)
            emit_mucopy(NCH - 2)
            emit_l8(NCH - 1)
            emit_mucopy(NCH - 1)

            nc.sync.dma_start(out=mu9_s[:], in_=mu9[8:8 + NDOSES, :])

    return _split_sync_waits(nc) if split_waits else nc


def _get_nc():
    if "nc" not in _NC_CACHE:
        _NC_CACHE["nc"] = build_nc()
    return _NC_CACHE["nc"]


def make_in_maps(inputs):
    idx = np.asarray(inputs["idx"], np.int64)
    tidx = np.asarray(inputs["tidx"], np.int64)
    cm = np.asarray(inputs["cell_map"]).astype(np.uint8)
    mi = np.asarray(inputs["is_missing"]).astype(np.uint8)
    dmv = np.asarray(inputs["drug_map"]).astype(np.uint8)
    cf = np.asarray(inputs["cell_features"], np.float32)
    me = np.asarray(inputs["missing_emb"], np.float32)
    de = np.asarray(inputs["drug_emb"], np.float32)
    W1 = np.asarray(inputs["W1"], np.float32)
    Wf1 = np.asarray(inputs["Wf1"], np.float32)
    Wf2 = np.asarray(inputs["Wf2"], np.float32)
    Wf3 = np.asarray(inputs["Wf3"], np.float32)
    b1 = np.asarray(inputs["b1"], np.float32)
    bf1 = np.asarray(inputs["bf1"], np.float32)
    bf2 = np.asarray(inputs["bf2"], np.float32)
    bf3 = np.asarray(inputs["bf3"], np.float32)

    # ---- bf16 blob ----
    bb = np.zeros((P, NBF), np_bf16)

    def put_bf(name, rows, arr):
        lo, hi = BF_L[name]
        bb[rows[0]:rows[1], lo:hi] = arr.astype(np_bf16)

    for kt in range(8):
        bb[:, BF_L["w1"][0] + kt * CEMB:BF_L["w1"][0] + (kt + 1) * CEMB] = \
            W1[kt * P:(kt + 1) * P, :].astype(np_bf16)
        bb[:, BF_L["wf1c"][0] + kt * HID:BF_L["wf1c"][0] + (kt + 1) * HID] = \
            Wf1[kt * P:(kt + 1) * P, :].astype(np_bf16)
        bb[:, BF_L["cft"][0] + kt * 100:BF_L["cft"][0] + (kt + 1) * 100] = \
            cf[:100, kt * P:(kt + 1) * P].T.astype(np_bf16)
    put_bf("wf1d", (0, DEMB), Wf1[CEMB:, :])
    put_bf("deT", (0, DEMB), de.T)
    put_bf("wf2a", (0, P), Wf2[0:P, :])
    put_bf("wf2b", (0, HID - P), Wf2[P:HID, :])
    w3p = Wf3[:, [1, 2, 3, 4, 5, 6, 7, 8, 0]]
    b3p = bf3[[1, 2, 3, 4, 5, 6, 7, 8, 0]]
    fma = np.concatenate([w3p[0:P, 0:8], np.tile(w3p[0:P, 8:9], (1, 9))], axis=1)
    put_bf("fma", (0, P), fma)
    fmb = np.zeros((P, 17), np.float32)
    fmb[0:HID - P, 0:8] = w3p[P:HID, 0:8]
    fmb[0:HID - P, 8:17] = np.tile(w3p[P:HID, 8:9], (1, 9))
    fmb[P - 1, 0:8] = b3p[0:8]
    fmb[P - 1, 8:17] = b3p[8]
    put_bf("fmb", (0, P), fmb)
    l8 = np.zeros((8, 17), np.float32)
    l8[:, 8:17] = np.triu(np.ones((8, NDOSES), np.float32), 1)
    put_bf("l8", (0, 8), l8)
    put_bf("ones512", (0, 1), np.ones((1, 512), np.float32))
    put_bf("grp_bc", (0, NG),
           np.array([[1.0 if (p // 16) == g else 0.0 for p in range(P)]
                     for g in range(NG)], np.float32))
    put_bf("grp_rd", (0, P),
           np.array([[1.0 if (p // 16) == g else 0.0 for g in range(NG)]
                     for p in range(P)], np.float32))
    sel = np.zeros((NG, NG * P), np.float32)
    for g in range(NG):
        sel[g, g * P:(g + 1) * P] = 1.0
    put_bf("selg", (0, NG), sel)

    # ---- f32 blob ----
    bf = np.zeros((P, NF32), np.float32)
    bf[0:100, F_L["meb"][0]:F_L["meb"][0] + CEMB] = me
    bf[0, F_L["b1r"][0]:F_L["b1r"][1]] = b1
    bf[0, F_L["bf1r"][0]:F_L["bf1r"][1]] = bf1
    bf[0, F_L["onesr"][0]:F_L["onesr"][1]] = 1.0
    bf[:, F_L["de"][0]:F_L["de"][0] + DEMB] = de[0:P, :]
    bf[:, F_L["de"][0] + DEMB:F_L["de"][0] + 2 * DEMB] = de[P:NDRUG, :]
    pp = np.arange(P)
    bf[:, F_L["qi"][0]] = pp % 16
    bf[:, F_L["ccl"][0]] = np.where(pp < 100, pp, pp + 28)
    bf[:, F_L["cch"][0]] = np.where(pp < HID - P, pp + 156, 1000)
    bf[:, F_L["cdl"][0]] = pp
    bf[:, F_L["cdh"][0]] = pp + P
    bf[:, F_L["bf2a"][0]] = bf2[0:P]
    bf[0:HID - P, F_L["bf2b"][0]] = bf2[P:HID]

    shared = dict(
        blob_bf=np.ascontiguousarray(bb),
        blob_f32=np.ascontiguousarray(bf),
        csmi=np.ascontiguousarray(cm | (mi << 7)),
        dm_map=np.ascontiguousarray(dmv),
    )

    def wrap16(vals):
        # vals [8192] in sample order k (g = k>>10, j = k&1023)
        # -> [128, 64] at [16g + (j & 15), j >> 4]
        v = vals.reshape(NG, GS // 16, 16)        # [g, j_hi, j_lo]
        v = np.transpose(v, (0, 2, 1))            # [g, j_lo, j_hi]
        return np.ascontiguousarray(v.reshape(P, GS // 16))

    in_maps = []
    for c in range(NCORES):
        ic = idx[c * BS:(c + 1) * BS]
        tc_ = tidx[c * BS:(c + 1) * BS]
        m = dict(shared)
        m["u_idx"] = wrap16((ic & (SLAB - 1)).astype(np.uint16))
        m["u_tidx"] = wrap16((tc_ & (SLAB - 1)).astype(np.uint16))
        m["q_idx"] = np.ascontiguousarray(
            (ic >> 14).astype(np_bf16).reshape(NG, GS))
        m["q_tidx"] = np.ascontiguousarray(
            (tc_ >> 14).astype(np_bf16).reshape(NG, GS))
        in_maps.append(m)
    return in_maps


def kernel(**inputs):
    nc = _get_nc()
    in_maps = make_in_maps(inputs)
    last_err = None
    for _attempt in range(3):
        try:
            res = run_bass_kernel_spmd(nc, in_maps, core_ids=list(range(NCORES)))
            return np.concatenate(
                [np.ascontiguousarray(res.results[c]["mu9_s"].T)
                 for c in range(NCORES)], axis=0)
        except Exception as e:  # wedged device sometimes recovers on retry
            last_err = e
    raise last_err


# revision 42
# speedup vs baseline: 1.2456x; 1.0349x over previous
"""Trainium2 Bass kernel for nn_DrugResponsePrior (embedding_lookup).

Spec guarantees: cell_map < 100, is_missing in {0,1}, drug_map < 256.  Each
row's result depends only on the cell state cs = cell_map[idx] +
100*is_missing[idx] (200 states) and dm = drug_map[tidx] (256 drugs).

Fully data-parallel (8 cores x 8192 samples, no collectives).  Per core:
  1. Host bit-packs csmi = cell_map | (is_missing << 7) (u8 - a pure bit
     repack; the state CODE cm + 128*mi is exact in bf16).  csmi/drug_map are
     loaded in a 16-slab SBUF layout (partition 16g+r holds entries
     [r*16384, (r+1)*16384) for every group g).
  2. Two GPSIMD indirect_copy gathers per 4096-sample piece fetch the 16
     slab candidates per sample; a one-hot mask over idx>>14 (grp_bc matmul +
     is_eq) and a group-reduce matmul produce v8 = per-sample code [8, 1024].
  3. Tables built once on device: A = l2n(cell emb) @ Wf1c + bf1 ([200,200])
     and Bd = l2n(drug_emb) @ Wf1d ([256,200]), bf16.
  4. Per 512-sample chunk: gpsimd partition_broadcast expands the codes to
     [128, 512]; two DVE is_eq (4x mode, bf16) build one-hot matrices; bf16
     matmuls run the MLP: h1 = relu(A^T Sc + Bd^T Sd), h2 = relu(Wf2^T h1 +
     bf2), fm = [fwd(1:9); mu-base] via one packed [.,17] lhsT, softplus on
     scalar engine, one L8 matmul accumulates the cumsum into the mu rows.
  Chunks are software-pipelined depth 3 so every engine streams without
  gaps (keeps the PE p-state at full clock).

All params ride in two packed blob tensors ([128, N] with large contiguous
partition lines) so the whole param load is 2 DMAs - the baseline's ~16k
small DMA descriptors were the main bottleneck.

All reference math runs on device; the host only reshapes/transposes/casts
inputs, bit-packs the two sub-byte index tables, and slices idx/tidx (pure
index arithmetic: & 16383, >> 14).
"""
import sys

if "/opt/trn_rl_repo" not in sys.path:
    sys.path.insert(0, "/opt/trn_rl_repo")

import numpy as np
import ml_dtypes

import concourse.bass as bass
import concourse.bass_isa as bass_isa
import concourse.mybir as mybir
import concourse.tile as tile
from concourse.bass_utils import run_bass_kernel_spmd

f32 = mybir.dt.float32
bf16 = mybir.dt.bfloat16
u16 = mybir.dt.uint16
u8 = mybir.dt.uint8
np_bf16 = ml_dtypes.bfloat16

B = 65536
R = 262144
NDRUG = 256
NFEAT = 1024
CEMB = 1024
DEMB = 128
HID = 200
NDOSES = 9
NCORES = 8

BS = B // NCORES            # 8192 samples per core
P = 128
NG = 8                      # gpsimd groups (16 partitions each)
GS = BS // NG               # 1024 samples per group
SLAB = R // 16              # 16384 entries per slab partition
EPS = 1e-12

_NC_CACHE = {}

# ---------------- packed blob layouts (host & device share these) ----------
def _layout(specs):
    out, off = {}, 0
    for name, n in specs:
        out[name] = (off, off + n)
        off += n
    return out, off

# bf16 param blobs, split by when the device needs them (3 staged DMAs)
T1_L, NT1 = _layout([
    ("w1", 8 * CEMB),        # 8 k-tiles [128, 1024]
    ("cft", 8 * 100),        # 8 k-tiles [128, 100]
])
T2_L, NT2 = _layout([
    ("wf1c", 8 * HID),       # 8 k-tiles [128, 200]
    ("wf1d", HID),           # [128, 200]
    ("deT", NDRUG),          # [128, 256] drug_emb^T
    ("meb", CEMB),           # rows 0:100 = missing_emb
    ("de", 2 * DEMB),        # drug_emb [256, 128] as two [128, 128] tiles
    ("grp_bc", P),           # rows 0:8: [g, p] = (p//16 == g)
    ("grp_rd", NG),          # [p, g] = (p//16 == g)
    ("selg", NG * P),        # block g: [g', p] = (g' == g)  (bcast lhsT)
])
T3_L, NT3 = _layout([
    ("wf2a", HID),           # Wf2[0:128, :]
    ("wf2b", HID),           # rows 0:72 = Wf2[128:200, :]
    ("fma", 17),             # [Wf3p[0:128, 0:8] | tile(col base, 9)]
    ("fmb", 17),             # rows 0:72 = Wf3p[128:200]; row 127 = biases
    ("l8", 17),              # rows 0:8: cols 0:8 zero, cols 8:17 (k < o)
    ("ones512", 512),        # row 0 = 1.0 (DMA'd to h2s1_st row 127)
])
# single-row f32 tensor (bias rows for matmul rhs; base partition 0)
BR_L, NBR = _layout([
    ("b1r", CEMB),
    ("bf1r", HID),
    ("onesr", P),
])
# per-partition f32 columns [128, NBC]
BC_L, NBC = _layout([
    ("qi", 1),               # p % 16
    ("ccl", 1),              # cs code, one-hot block lo
    ("cch", 1),              # cs code, block hi
    ("cdl", 1),              # dm code lo
    ("cdh", 1),              # dm code hi
    ("bf2a", 1),
    ("bf2b", 1),
])


def _split_sync_waits(nc, limit=1):
    """The walrus accepts at most one sync-wait per instruction; hoist excess
    waits onto same-engine NoOps inserted just before."""
    ctr = 0
    for bb in nc.main_func.blocks:
        new_list = []
        for inst in bb.instructions:
            si = inst.sync_info
            if si is not None and si.on_wait and len(si.on_wait) > limit:
                waits = list(si.on_wait)
                head, tail = waits[:-limit], waits[-limit:]
                for j in range(0, len(head), limit):
                    nop = mybir.InstNoOp(name=f"waitnop-{ctr}", engine=inst.engine)
                    ctr += 1
                    nop.sync_info = mybir.SyncInfo(
                        on_wait=list(head[j : j + limit]), on_update=[]
                    )
                    new_list.append(nop)
                inst.sync_info = mybir.SyncInfo(
                    on_wait=list(tail),
                    on_update=list(si.on_update) if si.on_update else [],
                )
            new_list.append(inst)
        bb.instructions[:] = new_list
    return nc


def build_nc(split_waits=True):
    nc = bass.Bass(num_devices=NCORES)
    AF = mybir.ActivationFunctionType
    ALU = mybir.AluOpType

    # ---------------- kernel I/O ----------------
    blob_t1 = nc.dram_tensor("blob_t1", [P, NT1], bf16, kind="ExternalInput")
    blob_t2 = nc.dram_tensor("blob_t2", [P, NT2], bf16, kind="ExternalInput")
    blob_t3 = nc.dram_tensor("blob_t3", [P, NT3], bf16, kind="ExternalInput")
    brow = nc.dram_tensor("brow", [1, NBR], f32, kind="ExternalInput")
    bcol = nc.dram_tensor("bcol", [P, NBC], f32, kind="ExternalInput")
    csmi = nc.dram_tensor("csmi", [R], u8, kind="ExternalInput")
    dm_map = nc.dram_tensor("dm_map", [R], u8, kind="ExternalInput")
    u_idx = nc.dram_tensor("u_idx", [P, GS // 16], u16, kind="ExternalInput")
    u_tidx = nc.dram_tensor("u_tidx", [P, GS // 16], u16, kind="ExternalInput")
    q_idx = nc.dram_tensor("q_idx", [NG, GS], bf16, kind="ExternalInput")
    q_tidx = nc.dram_tensor("q_tidx", [NG, GS], bf16, kind="ExternalInput")
    mu9_s = nc.dram_tensor("mu9_s", [NDOSES, BS], f32, kind="ExternalOutput")

    with tile.TileContext(nc) as tc, \
            tc.tile_pool(name="sbw", bufs=1) as sbw, \
            tc.tile_pool(name="sb", bufs=1) as sb:

        # ---- setup DMAs ----
        # queue A (sync -> DMA engines 0-7): index tensors + slabs
        # queue B (scalar -> DMA engines 8-15): param blobs, staged
        u_idx_sb = sb.tile([P, GS // 16], u16)
        u_tidx_sb = sb.tile([P, GS // 16], u16)
        q_idx_sb = sb.tile([NG, GS], bf16)
        q_tidx_sb = sb.tile([NG, GS], bf16)
        nc.sync.dma_start(out=u_idx_sb[:], in_=u_idx[:])
        nc.sync.dma_start(out=u_tidx_sb[:], in_=u_tidx[:])
        nc.sync.dma_start(out=q_idx_sb[:], in_=q_idx[:])
        nc.sync.dma_start(out=q_tidx_sb[:], in_=q_tidx[:])
        cs_slab = sbw.tile([P, SLAB], u8)
        dm_slab = sbw.tile([P, SLAB], u8)
        nc.sync.dma_start(out=cs_slab[:], in_=bass.AP(
            tensor=csmi.ap().tensor, offset=0,
            ap=[[0, NG], [SLAB, 16], [1, SLAB]]))
        nc.sync.dma_start(out=dm_slab[:], in_=bass.AP(
            tensor=dm_map.ap().tensor, offset=0,
            ap=[[0, NG], [SLAB, 16], [1, SLAB]]))
        bc_sb = sb.tile([P, NBC], f32)
        nc.sync.dma_start(out=bc_sb[:], in_=bcol[:])

        br_sb = sb.tile([1, NBR], f32)
        nc.scalar.dma_start(out=br_sb[:], in_=brow[:])
        t1_sb = sbw.tile([P, NT1], bf16)
        nc.scalar.dma_start(out=t1_sb[:], in_=blob_t1[:])
        t2_sb = sbw.tile([P, NT2], bf16)
        nc.scalar.dma_start(out=t2_sb[:], in_=blob_t2[:])
        t3_sb = sbw.tile([P, NT3], bf16)
        nc.scalar.dma_start(out=t3_sb[:], in_=blob_t3[:])

        # blob views
        me_sb = t2_sb[0:100, T2_L["meb"][0]:T2_L["meb"][1]]
        b1_row = br_sb[:, BR_L["b1r"][0]:BR_L["b1r"][1]]
        bf1_row = br_sb[:, BR_L["bf1r"][0]:BR_L["bf1r"][1]]
        ones100 = br_sb[:, BR_L["onesr"][0]:BR_L["onesr"][0] + 100]
        ones128 = br_sb[:, BR_L["onesr"][0]:BR_L["onesr"][1]]
        de0 = t2_sb[:, T2_L["de"][0]:T2_L["de"][0] + DEMB]
        de1 = t2_sb[:, T2_L["de"][0] + DEMB:T2_L["de"][0] + 2 * DEMB]
        qi_c = bc_sb[:, BC_L["qi"][0]:BC_L["qi"][1]]
        ccl_c = bc_sb[:, BC_L["ccl"][0]:BC_L["ccl"][1]]
        cch_c = bc_sb[:, BC_L["cch"][0]:BC_L["cch"][1]]
        cdl_c = bc_sb[:, BC_L["cdl"][0]:BC_L["cdl"][1]]
        cdh_c = bc_sb[:, BC_L["cdh"][0]:BC_L["cdh"][1]]
        bf2a_c = bc_sb[:, BC_L["bf2a"][0]:BC_L["bf2a"][1]]
        bf2b_c = bc_sb[0:72, BC_L["bf2b"][0]:BC_L["bf2b"][1]]
        w1_kt = [t1_sb[:, T1_L["w1"][0] + k * CEMB:T1_L["w1"][0] + (k + 1) * CEMB]
                 for k in range(8)]
        cft_kt = [t1_sb[:, T1_L["cft"][0] + k * 100:T1_L["cft"][0] + (k + 1) * 100]
                  for k in range(8)]
        wf1c_kt = [t2_sb[:, T2_L["wf1c"][0] + k * HID:T2_L["wf1c"][0] + (k + 1) * HID]
                   for k in range(8)]
        wf1d_sb = t2_sb[:, T2_L["wf1d"][0]:T2_L["wf1d"][1]]
        deT_sb = t2_sb[:, T2_L["deT"][0]:T2_L["deT"][1]]
        grp_bc = t2_sb[0:NG, T2_L["grp_bc"][0]:T2_L["grp_bc"][1]]
        grp_rd = t2_sb[:, T2_L["grp_rd"][0]:T2_L["grp_rd"][1]]
        selg = [t2_sb[0:NG, T2_L["selg"][0] + g * P:T2_L["selg"][0] + (g + 1) * P]
                for g in range(NG)]
        wf2a = t3_sb[:, T3_L["wf2a"][0]:T3_L["wf2a"][1]]
        wf2b = t3_sb[0:72, T3_L["wf2b"][0]:T3_L["wf2b"][1]]
        fma = t3_sb[:, T3_L["fma"][0]:T3_L["fma"][1]]
        fmb = t3_sb[:, T3_L["fmb"][0]:T3_L["fmb"][1]]
        l8_sb = t3_sb[0:8, T3_L["l8"][0]:T3_L["l8"][1]]

        # static h2 tiles (relu outputs; h2s1 rows 72:127 zero, row 127 ones
        # so fmb's row 127 supplies the biases)
        h2s0_st = sbw.tile([P, 512], bf16)
        h2s1_st = sbw.tile([P, 512], bf16)
        nc.vector.memset(h2s1_st[:], 0.0)
        nc.sync.dma_start(
            out=h2s1_st[P - 1:P, :],
            in_=blob_t3[0:1, T3_L["ones512"][0]:T3_L["ones512"][1]])

        # ======== table construction: A [200,200], Bd [256,200] (bf16) ======
        a_k = []
        bd_k = []
        with (
            tc.tile_pool(name="ps_tb", bufs=1, space="PSUM") as ps_tb,
            tc.tile_pool(name="ps_tr", bufs=3, space="PSUM") as ps_tr,
            tc.tile_pool(name="sbt", bufs=1) as sbt,
        ):
            from concourse.masks import make_identity
            ident = sbt.tile([P, P], bf16)
            make_identity(nc, ident[:])

            # P100 = relu(cf @ W1 + b1)  [100, 1024]
            p_sb = sbt.tile([100, CEMB], bf16)
            for nh in range(2):
                pps = ps_tb.tile([100, 512], f32, tag="pshard")
                for kt in range(8):
                    nc.tensor.matmul(
                        out=pps[:], lhsT=cft_kt[kt],
                        rhs=w1_kt[kt][:, nh * 512:(nh + 1) * 512],
                        start=(kt == 0), stop=False)
                nc.tensor.matmul(
                    out=pps[:], lhsT=ones100,
                    rhs=b1_row[:, nh * 512:(nh + 1) * 512], start=False, stop=True)
                nc.scalar.activation(
                    out=p_sb[:, nh * 512:(nh + 1) * 512], in_=pps[:], func=AF.Relu)

            # l2 norm scales for present / missing rows
            sq = sbt.tile([100, CEMB], f32)
            ssp = sbt.tile([100, 1], f32)
            ssm = sbt.tile([100, 1], f32)
            nc.scalar.activation(out=sq[:], in_=p_sb[:], func=AF.Square)
            nc.vector.reduce_sum(out=ssp[:], in_=sq[:], axis=mybir.AxisListType.X)
            nc.scalar.activation(out=sq[:], in_=me_sb, func=AF.Square)
            nc.vector.reduce_sum(out=ssm[:], in_=sq[:], axis=mybir.AxisListType.X)
            for ss in (ssp, ssm):
                nc.scalar.activation(out=ss[:], in_=ss[:], func=AF.Sqrt)
                nc.vector.tensor_scalar_max(out=ss[:], in0=ss[:], scalar1=EPS)
                nc.vector.reciprocal(out=ss[:], in_=ss[:])
            nc.vector.tensor_scalar_mul(out=p_sb[:], in0=p_sb[:], scalar1=ssp[:])
            nc.vector.tensor_scalar_mul(out=me_sb, in0=me_sb, scalar1=ssm[:])

            # CnT k-tiles [128, 200] bf16 (cols: 100 present + 100 missing)
            cnt_kt = []
            for kt in range(8):
                t = sbt.tile([P, 2 * 100], bf16, tag=f"cnt_{kt}")
                for (src, co) in ((p_sb[:], 0), (me_sb, 100)):
                    tp = ps_tr.tile([P, 100], bf16, tag="tr")
                    nc.tensor.transpose(
                        out=tp[:], in_=src[:, kt * P:(kt + 1) * P],
                        identity=ident[:100, :100])
                    nc.vector.tensor_copy(out=t[:, co:co + 100], in_=tp[:])
                cnt_kt.append(t)

            # A tiles (states on partitions): a_k[0] [128, 200], a_k[1] [72, 200]
            for (mt, msl) in ((0, slice(0, P)), (1, slice(P, HID))):
                mm = msl.stop - msl.start
                aps = ps_tb.tile([P, HID], f32, tag="a")
                for kt in range(8):
                    nc.tensor.matmul(
                        out=aps[:mm, :], lhsT=cnt_kt[kt][:, msl],
                        rhs=wf1c_kt[kt], start=(kt == 0), stop=False)
                nc.tensor.matmul(
                    out=aps[:mm, :], lhsT=ones128[:, :mm], rhs=bf1_row,
                    start=False, stop=True)
                t = sb.tile([mm, HID], bf16, tag=f"a_{mt}")
                nc.vector.tensor_copy(out=t[:], in_=aps[:mm, :])
                a_k.append(t)

            # drug tiles: per-drug l2 recip + Bd [128, 200] bf16 x2
            for (mt, de_p) in ((0, de0), (1, de1)):
                sqd = sbt.tile([P, DEMB], f32, tag="sqd")
                rd = sbt.tile([P, 1], f32, tag=f"rd_{mt}")
                nc.scalar.activation(out=sqd[:], in_=de_p, func=AF.Square)
                nc.vector.reduce_sum(out=rd[:], in_=sqd[:], axis=mybir.AxisListType.X)
                nc.scalar.activation(out=rd[:], in_=rd[:], func=AF.Sqrt)
                nc.vector.tensor_scalar_max(out=rd[:], in0=rd[:], scalar1=EPS)
                nc.vector.reciprocal(out=rd[:], in_=rd[:])
                bps = ps_tb.tile([P, HID], f32, tag="a")
                nc.tensor.matmul(out=bps[:], lhsT=deT_sb[:, mt * P:(mt + 1) * P],
                                 rhs=wf1d_sb, start=True, stop=True)
                t = sb.tile([P, HID], bf16, tag=f"bd_{mt}")
                nc.vector.tensor_scalar_mul(out=t[:], in0=bps[:], scalar1=rd[:])
                bd_k.append(t)

            # ======== lookup pieces (both up front) ========
            g_cs = sbt.tile([P, GS], u8)
            g_dm = sbt.tile([P, GS], u8)
            v8_cs = sb.tile([NG, GS], bf16)
            v8_dm = sb.tile([NG, GS], bf16)
            # cs gathers first: cs_slab's DMA lands before dm_slab's
            for t in range(2):
                nc.gpsimd.indirect_copy(
                    out=g_cs[:, t * 512:(t + 1) * 512].rearrange(
                        "p (n one) -> p n one", one=1),
                    data=cs_slab[:], idxs=u_idx_sb[:, t * 32:(t + 1) * 32],
                    i_know_ap_gather_is_preferred=True)
            for t in range(2):
                nc.gpsimd.indirect_copy(
                    out=g_dm[:, t * 512:(t + 1) * 512].rearrange(
                        "p (n one) -> p n one", one=1),
                    data=dm_slab[:], idxs=u_tidx_sb[:, t * 32:(t + 1) * 32],
                    i_know_ap_gather_is_preferred=True)
            for t in range(2):
                jsl = slice(t * 512, (t + 1) * 512)
                for (gt, qt, v8t) in ((g_cs, q_idx_sb, v8_cs),
                                      (g_dm, q_tidx_sb, v8_dm)):
                    qb = ps_tb.tile([P, 512], f32, tag="pqb")
                    nc.tensor.matmul(out=qb[:], lhsT=grp_bc, rhs=qt[:, jsl],
                                     start=True, stop=True)
                    qmask = sbt.tile([P, 512], bf16, tag="qmask")
                    nc.vector.tensor_scalar(
                        out=qmask[:], in0=qb[:], scalar1=qi_c, scalar2=None,
                        op0=ALU.is_equal)
                    gf = sbt.tile([P, 512], bf16, tag="gf")
                    nc.vector.tensor_copy(out=gf[:], in_=gt[:, jsl])
                    nc.vector.tensor_tensor(out=gf[:], in0=gf[:], in1=qmask[:],
                                            op=ALU.mult)
                    vpf = ps_tb.tile([NG, 512], f32, tag="pvpf")
                    nc.tensor.matmul(out=vpf[:], lhsT=grp_rd, rhs=gf[:],
                                     start=True, stop=True)
                    nc.vector.tensor_copy(out=v8t[:, jsl], in_=vpf[:])

        # ======== per-chunk pipeline ========
        chunks = [(g, pc) for pc in range(2) for g in range(NG)]
        NCH = len(chunks)

        with (
            tc.tile_pool(name="ps_h1", bufs=1, space="PSUM") as ps_h1,
            tc.tile_pool(name="ps_h2", bufs=1, space="PSUM") as ps_h2,
            tc.tile_pool(name="ps_fm", bufs=2, space="PSUM") as ps_fm,
            tc.tile_pool(name="ps_qb", bufs=1, space="PSUM") as ps_qb,
            tc.tile_pool(name="sbc", bufs=2) as sbc,
        ):
            bc_of, oh_of, h1ps_of, h1s_of, h2ps_of, fm_of, spb_of = \
                {}, {}, {}, {}, {}, {}, {}

            def emit_qb(i):
                # broadcast codes of chunk i's group to all 128 partitions
                g, pc = chunks[i]
                jsl = slice(pc * 512, (pc + 1) * 512)
                qbc = ps_qb.tile([P, 512], f32, tag="qbc")
                qbd = ps_qb.tile([P, 512], f32, tag="qbd")
                nc.tensor.matmul(out=qbc[:], lhsT=selg[g], rhs=v8_cs[:, jsl],
                                 start=True, stop=True)
                nc.tensor.matmul(out=qbd[:], lhsT=selg[g], rhs=v8_dm[:, jsl],
                                 start=True, stop=True)
                bc_of[i] = (qbc, qbd)

            def emit_bcopy(i):
                # bf16 SBUF copies so the is_eq runs in the DVE 4x mode;
                # cs on scalar, dm on vector to balance engine load
                qbc, qbd = bc_of.pop(i)
                bcc = sbc.tile([P, 512], bf16, tag="bcc")
                bcd = sbc.tile([P, 512], bf16, tag="bcd")
                nc.scalar.activation(out=bcc[:], in_=qbc[:], func=AF.Copy)
                nc.vector.tensor_copy(out=bcd[:], in_=qbd[:])
                bc_of[i] = (bcc, bcd)

            def emit_onehot(i):
                bcc, bcd = bc_of.pop(i)
                sc2 = sbc.tile([P, 1024], bf16, tag="sc2")
                sd2 = sbc.tile([P, 1024], bf16, tag="sd2")
                for (oh, bc, cl, ch_) in ((sc2, bcc, ccl_c, cch_c),
                                          (sd2, bcd, cdl_c, cdh_c)):
                    nc.vector.tensor_scalar(
                        out=oh[:, 0:512], in0=bc[:], scalar1=cl, scalar2=None,
                        op0=ALU.is_equal)
                    nc.vector.tensor_scalar(
                        out=oh[:, 512:1024], in0=bc[:], scalar1=ch_, scalar2=None,
                        op0=ALU.is_equal)
                oh_of[i] = (sc2, sd2)

            def emit_h1(i):
                sc2, sd2 = oh_of.pop(i)
                hps = []
                for (mt, msl) in ((0, slice(0, P)), (1, slice(P, HID))):
                    mm = msl.stop - msl.start
                    hp = ps_h1.tile([mm, 512], f32, tag=f"h1_{mt}")
                    nc.tensor.matmul(out=hp[:], lhsT=a_k[0][:, msl],
                                     rhs=sc2[:, 0:512], start=True, stop=False)
                    nc.tensor.matmul(out=hp[:], lhsT=a_k[1][:, msl],
                                     rhs=sc2[0:HID - P, 512:1024],
                                     start=False, stop=False)
                    nc.tensor.matmul(out=hp[:], lhsT=bd_k[0][:, msl],
                                     rhs=sd2[:, 0:512], start=False, stop=False)
                    nc.tensor.matmul(out=hp[:], lhsT=bd_k[1][:, msl],
                                     rhs=sd2[:, 512:1024], start=False, stop=True)
                    hps.append(hp)
                h1ps_of[i] = hps

            def emit_h1relu(i):
                hps = h1ps_of.pop(i)
                h1s = []
                for mt, hp in enumerate(hps):
                    mm = P if mt == 0 else HID - P
                    hs = sbc.tile([mm, 512], bf16, tag=f"h1s_{mt}")
                    nc.vector.tensor_scalar_max(out=hs[:], in0=hp[:], scalar1=0.0)
                    h1s.append(hs)
                h1s_of[i] = h1s

            def emit_h2(i):
                h1s = h1s_of.pop(i)
                hps = []
                for (mt, msl) in ((0, slice(0, P)), (1, slice(P, HID))):
                    mm = msl.stop - msl.start
                    hp = ps_h2.tile([mm, 512], f32, tag=f"h2_{mt}")
                    nc.tensor.matmul(out=hp[:], lhsT=wf2a[:, msl], rhs=h1s[0][:],
                                     start=True, stop=False)
                    nc.tensor.matmul(out=hp[:], lhsT=wf2b[:, msl], rhs=h1s[1][:],
                                     start=False, stop=True)
                    hps.append(hp)
                h2ps_of[i] = hps

            def emit_h2relu(i):
                hps = h2ps_of.pop(i)
                nc.scalar.activation(out=h2s0_st[:], in_=hps[0][:], func=AF.Relu,
                                     bias=bf2a_c, scale=1.0)
                nc.scalar.activation(out=h2s1_st[0:HID - P, :], in_=hps[1][:],
                                     func=AF.Relu, bias=bf2b_c, scale=1.0)

            def emit_fm(i):
                fm = ps_fm.tile([8 + NDOSES, 512], f32, tag="fm")
                nc.tensor.matmul(out=fm[:], lhsT=fma, rhs=h2s0_st[:],
                                 start=True, stop=False)
                nc.tensor.matmul(out=fm[:], lhsT=fmb, rhs=h2s1_st[:],
                                 start=False, stop=True)
                fm_of[i] = fm

            def emit_softplus(i):
                # softplus = ln(1 + exp(x)); Softplus isn't in CoreSim
                fm = fm_of[i]
                spb = sbc.tile([8, 512], bf16, tag="spb")
                nc.scalar.activation(out=spb[:], in_=fm[0:8, :], func=AF.Exp)
                nc.scalar.activation(out=spb[:], in_=spb[:], func=AF.Ln,
                                     bias=1.0, scale=1.0)
                spb_of[i] = spb

            def emit_l8(i):
                fm = fm_of[i]
                spb = spb_of.pop(i)
                nc.tensor.matmul(out=fm[:], lhsT=l8_sb, rhs=spb[:],
                                 start=False, stop=True, skip_group_check=True)

            def emit_mucopy(i):
                # rows 0:8 = spent f9 junk (not stored); rows 8:17 = mu
                g, pc = chunks[i]
                fm = fm_of.pop(i)
                n0 = g * GS + pc * 512
                muc = sbc.tile([8 + NDOSES, 512], f32, tag="muc")
                nc.vector.tensor_copy(out=muc[:], in_=fm[:])
                nc.sync.dma_start(out=mu9_s[:, n0:n0 + 512],
                                  in_=muc[8:8 + NDOSES, :])

            # prologue
            emit_qb(0)
            emit_bcopy(0)
            emit_onehot(0)

            for i in range(NCH):
                emit_h1(i)
                emit_h1relu(i)
                if i + 1 < NCH:
                    emit_qb(i + 1)
                    emit_bcopy(i + 1)
                    emit_onehot(i + 1)
                if i >= 1:
                    emit_h2(i - 1)
                    emit_h2relu(i - 1)
                    emit_fm(i - 1)
                    emit_softplus(i - 1)
                if i >= 2:
                    emit_l8(i - 2)
                    emit_mucopy(i - 2)
            for i in (NCH - 1,):
                emit_h2(i)
                emit_h2relu(i)
                emit_fm(i)
                emit_softplus(i)
            emit_l8(NCH - 2)
            emit_mucopy(NCH - 2)
            emit_l8(NCH - 1)
            emit_mucopy(NCH - 1)

    return _split_sync_waits(nc) if split_waits else nc


def _get_nc():
    if "nc" not in _NC_CACHE:
        _NC_CACHE["nc"] = build_nc()
    return _NC_CACHE["nc"]


def make_in_maps(inputs):
    idx = np.asarray(inputs["idx"], np.int64)
    tidx = np.asarray(inputs["tidx"], np.int64)
    cm = np.asarray(inputs["cell_map"]).astype(np.uint8)
    mi = np.asarray(inputs["is_missing"]).astype(np.uint8)
    dmv = np.asarray(inputs["drug_map"]).astype(np.uint8)
    cf = np.asarray(inputs["cell_features"], np.float32)
    me = np.asarray(inputs["missing_emb"], np.float32)
    de = np.asarray(inputs["drug_emb"], np.float32)
    W1 = np.asarray(inputs["W1"], np.float32)
    Wf1 = np.asarray(inputs["Wf1"], np.float32)
    Wf2 = np.asarray(inputs["Wf2"], np.float32)
    Wf3 = np.asarray(inputs["Wf3"], np.float32)
    b1 = np.asarray(inputs["b1"], np.float32)
    bf1 = np.asarray(inputs["bf1"], np.float32)
    bf2 = np.asarray(inputs["bf2"], np.float32)
    bf3 = np.asarray(inputs["bf3"], np.float32)

    # ---- bf16 blobs ----
    t1 = np.zeros((P, NT1), np_bf16)
    t2 = np.zeros((P, NT2), np_bf16)
    t3 = np.zeros((P, NT3), np_bf16)

    for kt in range(8):
        t1[:, T1_L["w1"][0] + kt * CEMB:T1_L["w1"][0] + (kt + 1) * CEMB] = \
            W1[kt * P:(kt + 1) * P, :].astype(np_bf16)
        t1[:, T1_L["cft"][0] + kt * 100:T1_L["cft"][0] + (kt + 1) * 100] = \
            cf[:100, kt * P:(kt + 1) * P].T.astype(np_bf16)
        t2[:, T2_L["wf1c"][0] + kt * HID:T2_L["wf1c"][0] + (kt + 1) * HID] = \
            Wf1[kt * P:(kt + 1) * P, :].astype(np_bf16)

    def put(blob, L, name, rows, arr):
        lo, hi = L[name]
        blob[rows[0]:rows[1], lo:hi] = arr.astype(np_bf16)

    put(t2, T2_L, "wf1d", (0, DEMB), Wf1[CEMB:, :])
    put(t2, T2_L, "deT", (0, DEMB), de.T)
    put(t2, T2_L, "meb", (0, 100), me)
    t2[:, T2_L["de"][0]:T2_L["de"][0] + DEMB] = de[0:P, :].astype(np_bf16)
    t2[:, T2_L["de"][0] + DEMB:T2_L["de"][0] + 2 * DEMB] = \
        de[P:NDRUG, :].astype(np_bf16)
    put(t2, T2_L, "grp_bc", (0, NG),
        np.array([[1.0 if (p // 16) == g else 0.0 for p in range(P)]
                  for g in range(NG)], np.float32))
    put(t2, T2_L, "grp_rd", (0, P),
        np.array([[1.0 if (p // 16) == g else 0.0 for g in range(NG)]
                  for p in range(P)], np.float32))
    sel = np.zeros((NG, NG * P), np.float32)
    for g in range(NG):
        sel[g, g * P:(g + 1) * P] = 1.0
    put(t2, T2_L, "selg", (0, NG), sel)

    put(t3, T3_L, "wf2a", (0, P), Wf2[0:P, :])
    put(t3, T3_L, "wf2b", (0, HID - P), Wf2[P:HID, :])
    w3p = Wf3[:, [1, 2, 3, 4, 5, 6, 7, 8, 0]]
    b3p = bf3[[1, 2, 3, 4, 5, 6, 7, 8, 0]]
    fma = np.concatenate([w3p[0:P, 0:8], np.tile(w3p[0:P, 8:9], (1, 9))], axis=1)
    put(t3, T3_L, "fma", (0, P), fma)
    fmb = np.zeros((P, 17), np.float32)
    fmb[0:HID - P, 0:8] = w3p[P:HID, 0:8]
    fmb[0:HID - P, 8:17] = np.tile(w3p[P:HID, 8:9], (1, 9))
    fmb[P - 1, 0:8] = b3p[0:8]
    fmb[P - 1, 8:17] = b3p[8]
    put(t3, T3_L, "fmb", (0, P), fmb)
    l8 = np.zeros((8, 17), np.float32)
    l8[:, 8:17] = np.triu(np.ones((8, NDOSES), np.float32), 1)
    put(t3, T3_L, "l8", (0, 8), l8)
    put(t3, T3_L, "ones512", (0, 1), np.ones((1, 512), np.float32))

    br = np.zeros((1, NBR), np.float32)
    br[0, BR_L["b1r"][0]:BR_L["b1r"][1]] = b1
    br[0, BR_L["bf1r"][0]:BR_L["bf1r"][1]] = bf1
    br[0, BR_L["onesr"][0]:BR_L["onesr"][1]] = 1.0

    bc = np.zeros((P, NBC), np.float32)
    pp = np.arange(P)
    bc[:, BC_L["qi"][0]] = pp % 16
    bc[:, BC_L["ccl"][0]] = np.where(pp < 100, pp, pp + 28)
    bc[:, BC_L["cch"][0]] = np.where(pp < HID - P, pp + 156, 1000)
    bc[:, BC_L["cdl"][0]] = pp
    bc[:, BC_L["cdh"][0]] = pp + P
    bc[:, BC_L["bf2a"][0]] = bf2[0:P]
    bc[0:HID - P, BC_L["bf2b"][0]] = bf2[P:HID]

    shared = dict(
        blob_t1=np.ascontiguousarray(t1),
        blob_t2=np.ascontiguousarray(t2),
        blob_t3=np.ascontiguousarray(t3),
        brow=np.ascontiguousarray(br),
        bcol=np.ascontiguousarray(bc),
        csmi=np.ascontiguousarray(cm | (mi << 7)),
        dm_map=np.ascontiguousarray(dmv),
    )

    def wrap16(vals):
        # vals [8192] in sample order k (g = k>>10, j = k&1023)
        # -> [128, 64] at [16g + (j & 15), j >> 4]
        v = vals.reshape(NG, GS // 16, 16)        # [g, j_hi, j_lo]
        v = np.transpose(v, (0, 2, 1))            # [g, j_lo, j_hi]
        return np.ascontiguousarray(v.reshape(P, GS // 16))

    in_maps = []
    for c in range(NCORES):
        ic = idx[c * BS:(c + 1) * BS]
        tc_ = tidx[c * BS:(c + 1) * BS]
        m = dict(shared)
        m["u_idx"] = wrap16((ic & (SLAB - 1)).astype(np.uint16))
        m["u_tidx"] = wrap16((tc_ & (SLAB - 1)).astype(np.uint16))
        m["q_idx"] = np.ascontiguousarray(
            (ic >> 14).astype(np_bf16).reshape(NG, GS))
        m["q_tidx"] = np.ascontiguousarray(
            (tc_ >> 14).astype(np_bf16).reshape(NG, GS))
        in_maps.append(m)
    return in_maps


def kernel(**inputs):
    nc = _get_nc()
    in_maps = make_in_maps(inputs)
    last_err = None
    for _attempt in range(3):
        try:
            res = run_bass_kernel_spmd(nc, in_maps, core_ids=list(range(NCORES)))
            return np.concatenate(
                [np.ascontiguousarray(res.results[c]["mu9_s"].T)
                 for c in range(NCORES)], axis=0)
        except Exception as e:  # wedged device sometimes recovers on retry
            last_err = e
    raise last_err


# revision 51
# speedup vs baseline: 1.5017x; 1.2056x over previous
"""Trainium2 Bass kernel for nn_DrugResponsePrior (embedding_lookup).

Spec guarantees: cell_map < 100, is_missing in {0,1}, drug_map < 256.  Each
row's result depends only on the cell state cs = cell_map[idx] +
100*is_missing[idx] (200 states) and dm = drug_map[tidx] (256 drugs).

Fully data-parallel (8 cores x 8192 samples, no collectives).  Per core:
  1. Host bit-packs csmi = cell_map | (is_missing << 7) (u8 - a pure bit
     repack; the state CODE cm + 128*mi is exact in bf16).  csmi/drug_map are
     loaded in a 16-slab SBUF layout (partition 16g+r holds entries
     [r*16384, (r+1)*16384) for every group g).
  2. Two GPSIMD indirect_copy gathers per 4096-sample piece fetch the 16
     slab candidates per sample; a one-hot mask over idx>>14 (grp_bc matmul +
     is_eq) and a group-reduce matmul produce v8 = per-sample code [8, 1024].
  3. Tables built once on device: A = l2n(cell emb) @ Wf1c + bf1 ([200,200])
     and Bd = l2n(drug_emb) @ Wf1d ([256,200]), bf16.
  4. Per 512-sample chunk: gpsimd partition_broadcast expands the codes to
     [128, 512]; two DVE is_eq (4x mode, bf16) build one-hot matrices; bf16
     matmuls run the MLP: h1 = relu(A^T Sc + Bd^T Sd), h2 = relu(Wf2^T h1 +
     bf2), fm = [fwd(1:9); mu-base] via one packed [.,17] lhsT, softplus on
     scalar engine, one L8 matmul accumulates the cumsum into the mu rows.
  Chunks are software-pipelined depth 3 so every engine streams without
  gaps (keeps the PE p-state at full clock).

All params ride in two packed blob tensors ([128, N] with large contiguous
partition lines) so the whole param load is 2 DMAs - the baseline's ~16k
small DMA descriptors were the main bottleneck.

All reference math runs on device; the host only reshapes/transposes/casts
inputs, bit-packs the two sub-byte index tables, and slices idx/tidx (pure
index arithmetic: & 16383, >> 14).
"""
import sys

if "/opt/trn_rl_repo" not in sys.path:
    sys.path.insert(0, "/opt/trn_rl_repo")

import numpy as np
import ml_dtypes

import concourse.bass as bass
import concourse.bass_isa as bass_isa
import concourse.mybir as mybir
import concourse.tile as tile
from concourse.bass_utils import run_bass_kernel_spmd

f32 = mybir.dt.float32
bf16 = mybir.dt.bfloat16
u16 = mybir.dt.uint16
u8 = mybir.dt.uint8
np_bf16 = ml_dtypes.bfloat16

B = 65536
R = 262144
NDRUG = 256
NFEAT = 1024
CEMB = 1024
DEMB = 128
HID = 200
NDOSES = 9
NCORES = 8

BS = B // NCORES            # 8192 samples per core
P = 128
NG = 8                      # gpsimd groups (16 partitions each)
GS = BS // NG               # 1024 samples per group
SLAB = R // 16              # 16384 entries per slab partition
EPS = 1e-12

_NC_CACHE = {}

# ---------------- packed blob layouts (host & device share these) ----------
def _layout(specs):
    out, off = {}, 0
    for name, n in specs:
        out[name] = (off, off + n)
        off += n
    return out, off

# bf16 param blobs, split by when the device needs them (3 staged DMAs)
T1_L, NT1 = _layout([
    ("w1", 8 * CEMB),        # 8 k-tiles [128, 1024]
    ("cft", 8 * 100),        # 8 k-tiles [128, 100]
])
T2_L, NT2 = _layout([
    ("wf1c", 8 * HID),       # 8 k-tiles [128, 200]
    ("wf1d", HID),           # [128, 200]
    ("deT", NDRUG),          # [128, 256] drug_emb^T
    ("meb", CEMB),           # rows 0:100 = missing_emb
    ("de", 2 * DEMB),        # drug_emb [256, 128] as two [128, 128] tiles
    ("grp_bc", P),           # rows 0:8: [g, p] = (p//16 == g)
    ("grp_rd", NG),          # [p, g] = (p//16 == g)
    ("selg", NG * P),        # block g: [g', p] = (g' == g)  (bcast lhsT)
])
T3_L, NT3 = _layout([
    ("wf2a", HID),           # Wf2[0:128, :]
    ("wf2b", HID),           # rows 0:72 = Wf2[128:200, :]
    ("fma", 17),             # [Wf3p[0:128, 0:8] | tile(col base, 9)]
    ("fmb", 17),             # rows 0:72 = Wf3p[128:200]; row 127 = biases
    ("l8", 17),              # rows 0:8: cols 0:8 zero, cols 8:17 (k < o)
    ("ones512", 512),        # row 0 = 1.0 (DMA'd to h2s1_st row 127)
])
# single-row f32 tensor (bias rows for matmul rhs; base partition 0)
BR_L, NBR = _layout([
    ("b1r", CEMB),
    ("bf1r", HID),
    ("onesr", P),
])
# per-partition f32 columns [128, NBC]
BC_L, NBC = _layout([
    ("qi", 1),               # p % 16
    ("ccl", 1),              # cs code, one-hot block lo
    ("cch", 1),              # cs code, block hi
    ("cdl", 1),              # dm code lo
    ("cdh", 1),              # dm code hi
    ("bf2a", 1),
    ("bf2b", 1),
])


def _split_sync_waits(nc, limit=1):
    """The walrus accepts at most one sync-wait per instruction; hoist excess
    waits onto same-engine NoOps inserted just before."""
    ctr = 0
    for bb in nc.main_func.blocks:
        new_list = []
        for inst in bb.instructions:
            si = inst.sync_info
            if si is not None and si.on_wait and len(si.on_wait) > limit:
                waits = list(si.on_wait)
                head, tail = waits[:-limit], waits[-limit:]
                for j in range(0, len(head), limit):
                    nop = mybir.InstNoOp(name=f"waitnop-{ctr}", engine=inst.engine)
                    ctr += 1
                    nop.sync_info = mybir.SyncInfo(
                        on_wait=list(head[j : j + limit]), on_update=[]
                    )
                    new_list.append(nop)
                inst.sync_info = mybir.SyncInfo(
                    on_wait=list(tail),
                    on_update=list(si.on_update) if si.on_update else [],
                )
            new_list.append(inst)
        bb.instructions[:] = new_list
    return nc


def build_nc(split_waits=True):
    nc = bass.Bass(num_devices=NCORES)
    AF = mybir.ActivationFunctionType
    ALU = mybir.AluOpType

    # ---------------- kernel I/O ----------------
    blob_t1 = nc.dram_tensor("blob_t1", [P, NT1], bf16, kind="ExternalInput")
    blob_t2 = nc.dram_tensor("blob_t2", [P, NT2], bf16, kind="ExternalInput")
    blob_t3 = nc.dram_tensor("blob_t3", [P, NT3], bf16, kind="ExternalInput")
    brow = nc.dram_tensor("brow", [1, NBR], f32, kind="ExternalInput")
    bcol = nc.dram_tensor("bcol", [P, NBC], f32, kind="ExternalInput")
    # host pre-replicates the 16-slab tables to all 8 groups (contiguous
    # DMA: ~266 GB/s vs ~80 GB/s for a stride-0 replicating read)
    csmi = nc.dram_tensor("csmi", [P, SLAB], u8, kind="ExternalInput")
    dm_map = nc.dram_tensor("dm_map", [P, SLAB], u8, kind="ExternalInput")
    u_idx = nc.dram_tensor("u_idx", [P, GS // 16], u16, kind="ExternalInput")
    u_tidx = nc.dram_tensor("u_tidx", [P, GS // 16], u16, kind="ExternalInput")
    q_idx = nc.dram_tensor("q_idx", [NG, GS], bf16, kind="ExternalInput")
    q_tidx = nc.dram_tensor("q_tidx", [NG, GS], bf16, kind="ExternalInput")
    mu9_s = nc.dram_tensor("mu9_s", [NDOSES, BS], f32, kind="ExternalOutput")

    with tile.TileContext(nc) as tc, \
            tc.tile_pool(name="sbw", bufs=1) as sbw, \
            tc.tile_pool(name="sb", bufs=1) as sb:

        # ---- setup DMAs ----
        # queue A (sync -> DMA engines 0-7): index tensors + slabs
        # queue B (scalar -> DMA engines 8-15): param blobs, staged
        u_idx_sb = sb.tile([P, GS // 16], u16)
        u_tidx_sb = sb.tile([P, GS // 16], u16)
        q_idx_sb = sb.tile([NG, GS], bf16)
        q_tidx_sb = sb.tile([NG, GS], bf16)
        nc.sync.dma_start(out=u_idx_sb[:], in_=u_idx[:])
        nc.sync.dma_start(out=u_tidx_sb[:], in_=u_tidx[:])
        nc.sync.dma_start(out=q_idx_sb[:], in_=q_idx[:])
        nc.sync.dma_start(out=q_tidx_sb[:], in_=q_tidx[:])
        bc_sb = sb.tile([P, NBC], f32)
        nc.sync.dma_start(out=bc_sb[:], in_=bcol[:])
        cs_slab = sbw.tile([P, SLAB], u8)
        dm_slab = sbw.tile([P, SLAB], u8)
        # A queue: cs table (gathers start ~15us), then tables blob
        nc.sync.dma_start(out=cs_slab[:], in_=csmi[:])
        t1_sb = sbw.tile([P, NT1], bf16)
        nc.sync.dma_start(out=t1_sb[:], in_=blob_t1[:])
        # B queue: small blobs, then dm table (needed only after cs gathers)
        br_sb = sb.tile([1, NBR], f32)
        nc.scalar.dma_start(out=br_sb[:], in_=brow[:])
        t2_sb = sbw.tile([P, NT2], bf16)
        nc.scalar.dma_start(out=t2_sb[:], in_=blob_t2[:])
        t3_sb = sbw.tile([P, NT3], bf16)
        nc.scalar.dma_start(out=t3_sb[:], in_=blob_t3[:])
        nc.scalar.dma_start(out=dm_slab[:], in_=dm_map[:])

        # blob views
        me_sb = t2_sb[0:100, T2_L["meb"][0]:T2_L["meb"][1]]
        b1_row = br_sb[:, BR_L["b1r"][0]:BR_L["b1r"][1]]
        bf1_row = br_sb[:, BR_L["bf1r"][0]:BR_L["bf1r"][1]]
        ones100 = br_sb[:, BR_L["onesr"][0]:BR_L["onesr"][0] + 100]
        ones128 = br_sb[:, BR_L["onesr"][0]:BR_L["onesr"][1]]
        de0 = t2_sb[:, T2_L["de"][0]:T2_L["de"][0] + DEMB]
        de1 = t2_sb[:, T2_L["de"][0] + DEMB:T2_L["de"][0] + 2 * DEMB]
        qi_c = bc_sb[:, BC_L["qi"][0]:BC_L["qi"][1]]
        ccl_c = bc_sb[:, BC_L["ccl"][0]:BC_L["ccl"][1]]
        cch_c = bc_sb[:, BC_L["cch"][0]:BC_L["cch"][1]]
        cdl_c = bc_sb[:, BC_L["cdl"][0]:BC_L["cdl"][1]]
        cdh_c = bc_sb[:, BC_L["cdh"][0]:BC_L["cdh"][1]]
        bf2a_c = bc_sb[:, BC_L["bf2a"][0]:BC_L["bf2a"][1]]
        bf2b_c = bc_sb[0:72, BC_L["bf2b"][0]:BC_L["bf2b"][1]]
        w1_kt = [t1_sb[:, T1_L["w1"][0] + k * CEMB:T1_L["w1"][0] + (k + 1) * CEMB]
                 for k in range(8)]
        cft_kt = [t1_sb[:, T1_L["cft"][0] + k * 100:T1_L["cft"][0] + (k + 1) * 100]
                  for k in range(8)]
        wf1c_kt = [t2_sb[:, T2_L["wf1c"][0] + k * HID:T2_L["wf1c"][0] + (k + 1) * HID]
                   for k in range(8)]
        wf1d_sb = t2_sb[:, T2_L["wf1d"][0]:T2_L["wf1d"][1]]
        deT_sb = t2_sb[:, T2_L["deT"][0]:T2_L["deT"][1]]
        grp_bc = t2_sb[0:NG, T2_L["grp_bc"][0]:T2_L["grp_bc"][1]]
        grp_rd = t2_sb[:, T2_L["grp_rd"][0]:T2_L["grp_rd"][1]]
        selg = [t2_sb[0:NG, T2_L["selg"][0] + g * P:T2_L["selg"][0] + (g + 1) * P]
                for g in range(NG)]
        wf2a = t3_sb[:, T3_L["wf2a"][0]:T3_L["wf2a"][1]]
        wf2b = t3_sb[0:72, T3_L["wf2b"][0]:T3_L["wf2b"][1]]
        fma = t3_sb[:, T3_L["fma"][0]:T3_L["fma"][1]]
        fmb = t3_sb[:, T3_L["fmb"][0]:T3_L["fmb"][1]]
        l8_sb = t3_sb[0:8, T3_L["l8"][0]:T3_L["l8"][1]]

        # static h2 tiles (relu outputs; h2s1 rows 72:127 zero, row 127 ones
        # so fmb's row 127 supplies the biases)
        h2s0_st = sbw.tile([P, 512], bf16)
        h2s1_st = sbw.tile([P, 512], bf16)
        nc.vector.memset(h2s1_st[:], 0.0)
        nc.sync.dma_start(
            out=h2s1_st[P - 1:P, :],
            in_=blob_t3[0:1, T3_L["ones512"][0]:T3_L["ones512"][1]])

        # lookup state (lives across the table scope and the chunk scope)
        g_cs = sb.tile([P, GS], u8)
        g_dm = sb.tile([P, GS], u8)
        v8_cs = sb.tile([NG, GS], bf16)
        v8_dm = sb.tile([NG, GS], bf16)

        def emit_resolve(t, mk_ps, names=("c", "d")):
            jsl = slice(t * 512, (t + 1) * 512)
            for (gt, qt, v8t, nm) in ((g_cs, q_idx_sb, v8_cs, "c"),
                                      (g_dm, q_tidx_sb, v8_dm, "d")):
                if nm not in names:
                    continue
                qb = mk_ps()
                nc.tensor.matmul(out=qb[:], lhsT=grp_bc, rhs=qt[:, jsl],
                                 start=True, stop=True)
                qmask = sb.tile([P, 512], bf16, tag=f"qmask_{nm}",
                                name=f"qmask_{nm}")
                nc.vector.tensor_scalar(
                    out=qmask[:], in0=qb[:], scalar1=qi_c, scalar2=None,
                    op0=ALU.is_equal)
                gf = sb.tile([P, 512], bf16, tag=f"gf_{nm}", name=f"gf_{nm}")
                nc.vector.tensor_copy(out=gf[:], in_=gt[:, jsl])
                nc.vector.tensor_tensor(out=gf[:], in0=gf[:], in1=qmask[:],
                                        op=ALU.mult)
                vpf = mk_ps()
                nc.tensor.matmul(out=vpf[0:NG, :], lhsT=grp_rd, rhs=gf[:],
                                 start=True, stop=True)
                nc.vector.tensor_copy(out=v8t[:, jsl], in_=vpf[0:NG, :])

        # ======== table construction: A [200,200], Bd [256,200] (bf16) ======
        a_k = []
        bd_k = []
        with (
            tc.tile_pool(name="ps_tb", bufs=1, space="PSUM") as ps_tb,
            tc.tile_pool(name="ps_tr", bufs=3, space="PSUM") as ps_tr,
            tc.tile_pool(name="sbt", bufs=1) as sbt,
        ):
            from concourse.masks import make_identity
            ident = sbt.tile([P, P], bf16)
            make_identity(nc, ident[:])

            # P100 = relu(cf @ W1 + b1)  [100, 1024]
            p_sb = sbt.tile([100, CEMB], bf16)
            for nh in range(2):
                pps = ps_tb.tile([100, 512], f32, tag="pshard")
                for kt in range(8):
                    nc.tensor.matmul(
                        out=pps[:], lhsT=cft_kt[kt],
                        rhs=w1_kt[kt][:, nh * 512:(nh + 1) * 512],
                        start=(kt == 0), stop=False)
                nc.tensor.matmul(
                    out=pps[:], lhsT=ones100,
                    rhs=b1_row[:, nh * 512:(nh + 1) * 512], start=False, stop=True)
                nc.scalar.activation(
                    out=p_sb[:, nh * 512:(nh + 1) * 512], in_=pps[:], func=AF.Relu)

            # l2 norm scales for present / missing rows
            sq = sbt.tile([100, CEMB], f32)
            ssp = sbt.tile([100, 1], f32)
            ssm = sbt.tile([100, 1], f32)
            nc.scalar.activation(out=sq[:], in_=p_sb[:], func=AF.Square)
            nc.vector.reduce_sum(out=ssp[:], in_=sq[:], axis=mybir.AxisListType.X)
            nc.scalar.activation(out=sq[:], in_=me_sb, func=AF.Square)
            nc.vector.reduce_sum(out=ssm[:], in_=sq[:], axis=mybir.AxisListType.X)
            for ss in (ssp, ssm):
                nc.scalar.activation(out=ss[:], in_=ss[:], func=AF.Sqrt)
                nc.vector.tensor_scalar_max(out=ss[:], in0=ss[:], scalar1=EPS)
                nc.vector.reciprocal(out=ss[:], in_=ss[:])
            nc.vector.tensor_scalar_mul(out=p_sb[:], in0=p_sb[:], scalar1=ssp[:])
            nc.vector.tensor_scalar_mul(out=me_sb, in0=me_sb, scalar1=ssm[:])

            # CnT k-tiles [128, 200] bf16 (cols: 100 present + 100 missing)
            cnt_kt = []
            for kt in range(8):
                t = sbt.tile([P, 2 * 100], bf16, tag=f"cnt_{kt}")
                for (src, co) in ((p_sb[:], 0), (me_sb, 100)):
                    tp = ps_tr.tile([P, 100], bf16, tag="tr")
                    nc.tensor.transpose(
                        out=tp[:], in_=src[:, kt * P:(kt + 1) * P],
                        identity=ident[:100, :100])
                    nc.vector.tensor_copy(out=t[:, co:co + 100], in_=tp[:])
                cnt_kt.append(t)

            # A tiles (states on partitions): a_k[0] [128, 200], a_k[1] [72, 200]
            for (mt, msl) in ((0, slice(0, P)), (1, slice(P, HID))):
                mm = msl.stop - msl.start
                aps = ps_tb.tile([P, HID], f32, tag="a")
                for kt in range(8):
                    nc.tensor.matmul(
                        out=aps[:mm, :], lhsT=cnt_kt[kt][:, msl],
                        rhs=wf1c_kt[kt], start=(kt == 0), stop=False)
                nc.tensor.matmul(
                    out=aps[:mm, :], lhsT=ones128[:, :mm], rhs=bf1_row,
                    start=False, stop=True)
                t = sb.tile([mm, HID], bf16, tag=f"a_{mt}")
                nc.vector.tensor_copy(out=t[:], in_=aps[:mm, :])
                a_k.append(t)

            # drug tiles: per-drug l2 recip + Bd [128, 200] bf16 x2
            for (mt, de_p) in ((0, de0), (1, de1)):
                sqd = sbt.tile([P, DEMB], f32, tag="sqd")
                rd = sbt.tile([P, 1], f32, tag=f"rd_{mt}")
                nc.scalar.activation(out=sqd[:], in_=de_p, func=AF.Square)
                nc.vector.reduce_sum(out=rd[:], in_=sqd[:], axis=mybir.AxisListType.X)
                nc.scalar.activation(out=rd[:], in_=rd[:], func=AF.Sqrt)
                nc.vector.tensor_scalar_max(out=rd[:], in0=rd[:], scalar1=EPS)
                nc.vector.reciprocal(out=rd[:], in_=rd[:])
                bps = ps_tb.tile([P, HID], f32, tag="a")
                nc.tensor.matmul(out=bps[:], lhsT=deT_sb[:, mt * P:(mt + 1) * P],
                                 rhs=wf1d_sb, start=True, stop=True)
                t = sb.tile([P, HID], bf16, tag=f"bd_{mt}")
                nc.vector.tensor_scalar_mul(out=t[:], in0=bps[:], scalar1=rd[:])
                bd_k.append(t)

            # ======== lookup gathers (gpsimd runs these back to back) ========
            for t in range(2):
                nc.gpsimd.indirect_copy(
                    out=g_cs[:, t * 512:(t + 1) * 512].rearrange(
                        "p (n one) -> p n one", one=1),
                    data=cs_slab[:], idxs=u_idx_sb[:, t * 32:(t + 1) * 32],
                    i_know_ap_gather_is_preferred=True)
                nc.gpsimd.indirect_copy(
                    out=g_dm[:, t * 512:(t + 1) * 512].rearrange(
                        "p (n one) -> p n one", one=1),
                    data=dm_slab[:], idxs=u_tidx_sb[:, t * 32:(t + 1) * 32],
                    i_know_ap_gather_is_preferred=True)
            # piece 0 resolved here; piece 1 resolved mid-chunk-stream so the
            # engine FIFOs don't head-of-line block on its gathers
            emit_resolve(0, lambda: ps_tb.tile([P, 512], f32, tag="pqb",
                                               name="pqb"))

        # ======== per-chunk pipeline ========
        chunks = [(g, pc) for pc in range(2) for g in range(NG)]
        NCH = len(chunks)

        with (
            tc.tile_pool(name="ps_h1", bufs=1, space="PSUM") as ps_h1,
            tc.tile_pool(name="ps_h2", bufs=1, space="PSUM") as ps_h2,
            tc.tile_pool(name="ps_fm", bufs=2, space="PSUM") as ps_fm,
            tc.tile_pool(name="ps_qb", bufs=1, space="PSUM") as ps_qb,
            tc.tile_pool(name="sbc", bufs=2) as sbc,
        ):
            bc_of, oh_of, h1ps_of, h1s_of, h2ps_of, fm_of, spb_of = \
                {}, {}, {}, {}, {}, {}, {}

            def emit_qb(i):
                # broadcast codes of chunk i's group to all 128 partitions
                g, pc = chunks[i]
                jsl = slice(pc * 512, (pc + 1) * 512)
                qbc = ps_qb.tile([P, 512], f32, tag="qbc")
                qbd = ps_qb.tile([P, 512], f32, tag="qbd")
                nc.tensor.matmul(out=qbc[:], lhsT=selg[g], rhs=v8_cs[:, jsl],
                                 start=True, stop=True)
                nc.tensor.matmul(out=qbd[:], lhsT=selg[g], rhs=v8_dm[:, jsl],
                                 start=True, stop=True)
                bc_of[i] = (qbc, qbd)

            def emit_bcopy(i):
                # bf16 SBUF copies so the is_eq runs in the DVE 4x mode;
                # cs on scalar, dm on vector to balance engine load
                qbc, qbd = bc_of.pop(i)
                bcc = sbc.tile([P, 512], bf16, tag="bcc")
                bcd = sbc.tile([P, 512], bf16, tag="bcd")
                nc.scalar.activation(out=bcc[:], in_=qbc[:], func=AF.Copy)
                nc.vector.tensor_copy(out=bcd[:], in_=qbd[:])
                bc_of[i] = (bcc, bcd)

            def emit_onehot(i):
                bcc, bcd = bc_of.pop(i)
                sc2 = sbc.tile([P, 1024], bf16, tag="sc2")
                sd2 = sbc.tile([P, 1024], bf16, tag="sd2")
                for (oh, bc, cl, ch_) in ((sc2, bcc, ccl_c, cch_c),
                                          (sd2, bcd, cdl_c, cdh_c)):
                    nc.vector.tensor_scalar(
                        out=oh[:, 0:512], in0=bc[:], scalar1=cl, scalar2=None,
                        op0=ALU.is_equal)
                    nc.vector.tensor_scalar(
                        out=oh[:, 512:1024], in0=bc[:], scalar1=ch_, scalar2=None,
                        op0=ALU.is_equal)
                oh_of[i] = (sc2, sd2)

            def emit_h1(i):
                sc2, sd2 = oh_of.pop(i)
                hps = []
                for (mt, msl) in ((0, slice(0, P)), (1, slice(P, HID))):
                    mm = msl.stop - msl.start
                    hp = ps_h1.tile([mm, 512], f32, tag=f"h1_{mt}")
                    nc.tensor.matmul(out=hp[:], lhsT=a_k[0][:, msl],
                                     rhs=sc2[:, 0:512], start=True, stop=False)
                    nc.tensor.matmul(out=hp[:], lhsT=a_k[1][:, msl],
                                     rhs=sc2[0:HID - P, 512:1024],
                                     start=False, stop=False)
                    nc.tensor.matmul(out=hp[:], lhsT=bd_k[0][:, msl],
                                     rhs=sd2[:, 0:512], start=False, stop=False)
                    nc.tensor.matmul(out=hp[:], lhsT=bd_k[1][:, msl],
                                     rhs=sd2[:, 512:1024], start=False, stop=True)
                    hps.append(hp)
                h1ps_of[i] = hps

            def emit_h1relu(i):
                hps = h1ps_of.pop(i)
                h1s = []
                for mt, hp in enumerate(hps):
                    mm = P if mt == 0 else HID - P
                    hs = sbc.tile([mm, 512], bf16, tag=f"h1s_{mt}")
                    nc.vector.tensor_scalar_max(out=hs[:], in0=hp[:], scalar1=0.0)
                    h1s.append(hs)
                h1s_of[i] = h1s

            def emit_h2(i):
                h1s = h1s_of.pop(i)
                hps = []
                for (mt, msl) in ((0, slice(0, P)), (1, slice(P, HID))):
                    mm = msl.stop - msl.start
                    hp = ps_h2.tile([mm, 512], f32, tag=f"h2_{mt}")
                    nc.tensor.matmul(out=hp[:], lhsT=wf2a[:, msl], rhs=h1s[0][:],
                                     start=True, stop=False)
                    nc.tensor.matmul(out=hp[:], lhsT=wf2b[:, msl], rhs=h1s[1][:],
                                     start=False, stop=True)
                    hps.append(hp)
                h2ps_of[i] = hps

            def emit_h2relu(i):
                hps = h2ps_of.pop(i)
                nc.scalar.activation(out=h2s0_st[:], in_=hps[0][:], func=AF.Relu,
                                     bias=bf2a_c, scale=1.0)
                nc.scalar.activation(out=h2s1_st[0:HID - P, :], in_=hps[1][:],
                                     func=AF.Relu, bias=bf2b_c, scale=1.0)

            def emit_fm(i):
                fm = ps_fm.tile([8 + NDOSES, 512], f32, tag="fm")
                nc.tensor.matmul(out=fm[:], lhsT=fma, rhs=h2s0_st[:],
                                 start=True, stop=False)
                nc.tensor.matmul(out=fm[:], lhsT=fmb, rhs=h2s1_st[:],
                                 start=False, stop=True)
                fm_of[i] = fm

            def emit_softplus(i):
                # softplus = ln(1 + exp(x)); Softplus isn't in CoreSim
                fm = fm_of[i]
                spb = sbc.tile([8, 512], bf16, tag="spb")
                nc.scalar.activation(out=spb[:], in_=fm[0:8, :], func=AF.Exp)
                nc.scalar.activation(out=spb[:], in_=spb[:], func=AF.Ln,
                                     bias=1.0, scale=1.0)
                spb_of[i] = spb

            def emit_l8(i):
                fm = fm_of[i]
                spb = spb_of.pop(i)
                nc.tensor.matmul(out=fm[:], lhsT=l8_sb, rhs=spb[:],
                                 start=False, stop=True, skip_group_check=True)

            def emit_mucopy(i):
                # rows 0:8 = spent f9 junk (not stored); rows 8:17 = mu
                g, pc = chunks[i]
                fm = fm_of.pop(i)
                n0 = g * GS + pc * 512
                muc = sbc.tile([8 + NDOSES, 512], f32, tag="muc")
                nc.vector.tensor_copy(out=muc[:], in_=fm[:])
                nc.sync.dma_start(out=mu9_s[:, n0:n0 + 512],
                                  in_=muc[8:8 + NDOSES, :])

            # prologue
            emit_qb(0)
            emit_bcopy(0)
            emit_onehot(0)

            mk_prs = lambda: ps_h1.tile([P, 512], f32, tag="h1_0", name="prs")
            for i in range(NCH):
                emit_h1(i)
                emit_h1relu(i)
                if i == NCH // 2 - 2:
                    # resolve piece 1 cs (its gather lands around now); the
                    # h1_0 bank's next chunk use is pc-1 (needs this anyway)
                    emit_resolve(1, mk_prs, names=("c",))
                if i + 1 < NCH and i != NCH // 2 - 1:
                    emit_qb(i + 1)
                    emit_bcopy(i + 1)
                    emit_onehot(i + 1)
                if i >= 1:
                    emit_h2(i - 1)
                    emit_h2relu(i - 1)
                    emit_fm(i - 1)
                    emit_softplus(i - 1)
                if i == NCH // 2 - 1:
                    # piece-1 dm resolve late in the iteration, then the
                    # deferred qb for the first pc-1 chunk
                    emit_resolve(1, mk_prs, names=("d",))
                    emit_qb(i + 1)
                    emit_bcopy(i + 1)
                    emit_onehot(i + 1)
                if i >= 2:
                    emit_l8(i - 2)
                    emit_mucopy(i - 2)
            for i in (NCH - 1,):
                emit_h2(i)
                emit_h2relu(i)
                emit_fm(i)
                emit_softplus(i)
            emit_l8(NCH - 2)
            emit_mucopy(NCH - 2)
            emit_l8(NCH - 1)
            emit_mucopy(NCH - 1)

    return _split_sync_waits(nc) if split_waits else nc


def _get_nc():
    if "nc" not in _NC_CACHE:
        _NC_CACHE["nc"] = build_nc()
    return _NC_CACHE["nc"]


def make_in_maps(inputs):
    idx = np.asarray(inputs["idx"], np.int64)
    tidx = np.asarray(inputs["tidx"], np.int64)
    cm = np.asarray(inputs["cell_map"]).astype(np.uint8)
    mi = np.asarray(inputs["is_missing"]).astype(np.uint8)
    dmv = np.asarray(inputs["drug_map"]).astype(np.uint8)
    cf = np.asarray(inputs["cell_features"], np.float32)
    me = np.asarray(inputs["missing_emb"], np.float32)
    de = np.asarray(inputs["drug_emb"], np.float32)
    W1 = np.asarray(inputs["W1"], np.float32)
    Wf1 = np.asarray(inputs["Wf1"], np.float32)
    Wf2 = np.asarray(inputs["Wf2"], np.float32)
    Wf3 = np.asarray(inputs["Wf3"], np.float32)
    b1 = np.asarray(inputs["b1"], np.float32)
    bf1 = np.asarray(inputs["bf1"], np.float32)
    bf2 = np.asarray(inputs["bf2"], np.float32)
    bf3 = np.asarray(inputs["bf3"], np.float32)

    # ---- bf16 blobs ----
    t1 = np.zeros((P, NT1), np_bf16)
    t2 = np.zeros((P, NT2), np_bf16)
    t3 = np.zeros((P, NT3), np_bf16)

    for kt in range(8):
        t1[:, T1_L["w1"][0] + kt * CEMB:T1_L["w1"][0] + (kt + 1) * CEMB] = \
            W1[kt * P:(kt + 1) * P, :].astype(np_bf16)
        t1[:, T1_L["cft"][0] + kt * 100:T1_L["cft"][0] + (kt + 1) * 100] = \
            cf[:100, kt * P:(kt + 1) * P].T.astype(np_bf16)
        t2[:, T2_L["wf1c"][0] + kt * HID:T2_L["wf1c"][0] + (kt + 1) * HID] = \
            Wf1[kt * P:(kt + 1) * P, :].astype(np_bf16)

    def put(blob, L, name, rows, arr):
        lo, hi = L[name]
        blob[rows[0]:rows[1], lo:hi] = arr.astype(np_bf16)

    put(t2, T2_L, "wf1d", (0, DEMB), Wf1[CEMB:, :])
    put(t2, T2_L, "deT", (0, DEMB), de.T)
    put(t2, T2_L, "meb", (0, 100), me)
    t2[:, T2_L["de"][0]:T2_L["de"][0] + DEMB] = de[0:P, :].astype(np_bf16)
    t2[:, T2_L["de"][0] + DEMB:T2_L["de"][0] + 2 * DEMB] = \
        de[P:NDRUG, :].astype(np_bf16)
    put(t2, T2_L, "grp_bc", (0, NG),
        np.array([[1.0 if (p // 16) == g else 0.0 for p in range(P)]
                  for g in range(NG)], np.float32))
    put(t2, T2_L, "grp_rd", (0, P),
        np.array([[1.0 if (p // 16) == g else 0.0 for g in range(NG)]
                  for p in range(P)], np.float32))
    sel = np.zeros((NG, NG * P), np.float32)
    for g in range(NG):
        sel[g, g * P:(g + 1) * P] = 1.0
    put(t2, T2_L, "selg", (0, NG), sel)

    put(t3, T3_L, "wf2a", (0, P), Wf2[0:P, :])
    put(t3, T3_L, "wf2b", (0, HID - P), Wf2[P:HID, :])
    w3p = Wf3[:, [1, 2, 3, 4, 5, 6, 7, 8, 0]]
    b3p = bf3[[1, 2, 3, 4, 5, 6, 7, 8, 0]]
    fma = np.concatenate([w3p[0:P, 0:8], np.tile(w3p[0:P, 8:9], (1, 9))], axis=1)
    put(t3, T3_L, "fma", (0, P), fma)
    fmb = np.zeros((P, 17), np.float32)
    fmb[0:HID - P, 0:8] = w3p[P:HID, 0:8]
    fmb[0:HID - P, 8:17] = np.tile(w3p[P:HID, 8:9], (1, 9))
    fmb[P - 1, 0:8] = b3p[0:8]
    fmb[P - 1, 8:17] = b3p[8]
    put(t3, T3_L, "fmb", (0, P), fmb)
    l8 = np.zeros((8, 17), np.float32)
    l8[:, 8:17] = np.triu(np.ones((8, NDOSES), np.float32), 1)
    put(t3, T3_L, "l8", (0, 8), l8)
    put(t3, T3_L, "ones512", (0, 1), np.ones((1, 512), np.float32))

    br = np.zeros((1, NBR), np.float32)
    br[0, BR_L["b1r"][0]:BR_L["b1r"][1]] = b1
    br[0, BR_L["bf1r"][0]:BR_L["bf1r"][1]] = bf1
    br[0, BR_L["onesr"][0]:BR_L["onesr"][1]] = 1.0

    bc = np.zeros((P, NBC), np.float32)
    pp = np.arange(P)
    bc[:, BC_L["qi"][0]] = pp % 16
    bc[:, BC_L["ccl"][0]] = np.where(pp < 100, pp, pp + 28)
    bc[:, BC_L["cch"][0]] = np.where(pp < HID - P, pp + 156, 1000)
    bc[:, BC_L["cdl"][0]] = pp
    bc[:, BC_L["cdh"][0]] = pp + P
    bc[:, BC_L["bf2a"][0]] = bf2[0:P]
    bc[0:HID - P, BC_L["bf2b"][0]] = bf2[P:HID]

    shared = dict(
        blob_t1=np.ascontiguousarray(t1),
        blob_t2=np.ascontiguousarray(t2),
        blob_t3=np.ascontiguousarray(t3),
        brow=np.ascontiguousarray(br),
        bcol=np.ascontiguousarray(bc),
        # slab layout replicated to all 8 groups on the host (layout-only op)
        csmi=np.ascontiguousarray(
            np.tile((cm | (mi << 7)).reshape(16, SLAB), (NG, 1))),
        dm_map=np.ascontiguousarray(np.tile(dmv.reshape(16, SLAB), (NG, 1))),
    )

    def wrap16(vals):
        # vals [8192] in sample order k (g = k>>10, j = k&1023)
        # -> [128, 64] at [16g + (j & 15), j >> 4]
        v = vals.reshape(NG, GS // 16, 16)        # [g, j_hi, j_lo]
        v = np.transpose(v, (0, 2, 1))            # [g, j_lo, j_hi]
        return np.ascontiguousarray(v.reshape(P, GS // 16))

    in_maps = []
    for c in range(NCORES):
        ic = idx[c * BS:(c + 1) * BS]
        tc_ = tidx[c * BS:(c + 1) * BS]
        m = dict(shared)
        m["u_idx"] = wrap16((ic & (SLAB - 1)).astype(np.uint16))
        m["u_tidx"] = wrap16((tc_ & (SLAB - 1)).astype(np.uint16))
        m["q_idx"] = np.ascontiguousarray(
            (ic >> 14).astype(np_bf16).reshape(NG, GS))
        m["q_tidx"] = np.ascontiguousarray(
            (tc_ >> 14).astype(np_bf16).reshape(NG, GS))
        in_maps.append(m)
    return in_maps


def kernel(**inputs):
    nc = _get_nc()
    in_maps = make_in_maps(inputs)
    last_err = None
    for _attempt in range(3):
        try:
            res = run_bass_kernel_spmd(nc, in_maps, core_ids=list(range(NCORES)))
            return np.concatenate(
                [np.ascontiguousarray(res.results[c]["mu9_s"].T)
                 for c in range(NCORES)], axis=0)
        except Exception as e:  # wedged device sometimes recovers on retry
            last_err = e
    raise last_err
